# revision 9
# baseline (speedup 1.0000x reference)
"""Trainium2 Bass kernel for the spiking spectral net (nn_ASFF).

Pipeline: LIF -> FFT2 -> LIF -> blockdiag matmul -> BN -> LIF -> blockdiag
matmul -> BN -> combine -> softshrink -> FFT2.real -> BN.

Sharding: data-parallel over B (8 samples -> 8 cores). Four SPMD NEFF passes
with host-side all-reduce of BatchNorm statistics between them (stats are
[C]-vectors; everything heavy stays on device).

Layout notes:
 - c-layout: [128 partitions = half of C, 2304 free = (u,v) flattened hw]
 - spatial layout: [96 partitions = (d,h) with d = C-half, 6144 free]
 - FFT2 per 48x48 tile is done as W-side DFT (contract w), TensorE
   transpose, H-side DFT (contract h). DFT matrices are symmetric.
 - Matmuls on spike inputs use bf16 hi/lo-split DFT/weight matrices
   (exact to ~2^-17); the second FFT side has continuous input and uses
   exact fp32 matmuls. Post-threshold math (pass 3) is bf16 throughout.
"""
import sys
sys.path.insert(0, '/opt/trn_rl_repo')
import numpy as np
import ml_dtypes
import concourse.bass as bass
import concourse.tile as tile
import concourse.mybir as mybir
from concourse.bass_utils import run_bass_kernel_spmd

f32, bf16, f32r = mybir.dt.float32, mybir.dt.bfloat16, mybir.dt.float32r
AOT = mybir.AluOpType
AFT = mybir.ActivationFunctionType

T, B, C, H, W = 4, 8, 256, 48, 48
K, BS = 8, 32
HW = H * W            # 2304
NCORES = 8
NP = 96               # spatial-layout partitions (2 c-halves x 48)
NF = 6144             # spatial-layout free size (64 pairs x 2 x 48)
LAM = 0.06
EPS = 1e-5
NCHUNK = [480] * 12 + [384]           # spatial free chunking (5/4 pairs each)
CCHUNK = [512, 512, 512, 512, 256]    # c-layout free chunking of 2304

_BF = ml_dtypes.bfloat16
SBIG = float(2 ** 30)


def _hilo(x):
    hi = x.astype(np.float32).astype(_BF)
    lo = (x.astype(np.float32) - hi.astype(np.float32)).astype(_BF)
    return hi, lo


def _dft():
    j = np.arange(48)
    ang = -2.0 * np.pi * np.outer(j, j) / 48.0
    fr = (np.cos(ang) / np.sqrt(48.0)).astype(np.float32)
    fi = (np.sin(ang) / np.sqrt(48.0)).astype(np.float32)
    return fr, fi


def _diag2(m):
    out = np.zeros((96, 96), m.dtype)
    out[:48, :48] = m
    out[48:, 48:] = m
    return out


def split_waits(nc, max_waits=1):
    """This toolchain's walrus only tolerates one sync-wait per instruction;
    spill extra waits onto NoOps inserted just before the instruction."""
    ctr = 0
    for f in nc.m.functions:
        for bb in f.blocks:
            insts = list(bb.instructions)
            out = []
            changed = False
            for inst in insts:
                si = inst.sync_info
                waits = list(si.on_wait) if si else []
                if len(waits) > max_waits:
                    for wcond in waits[:-max_waits]:
                        ctr += 1
                        nop = mybir.InstNoOp(name=f"wsplit-{ctr}")
                        nop.engine = inst.engine
                        nop.sync_info = mybir.SyncInfo(on_wait=[wcond], on_update=[])
                        out.append(nop)
                    si.on_wait = waits[-max_waits:]
                    changed = True
                out.append(inst)
            if changed:
                bb.instructions = out
    return ctr


def _lif_step(nc, w_state, x_ap, s_out, ns_scratch=None):
    """One LIF step on w = 2*v scaled state: u = 0.5*w + x (into w_state),
    s = (u >= 1), w = min(u,1) - s  (== u*(u<1) bitwise).  x_ap may be PSUM."""
    nc.vector.scalar_tensor_tensor(w_state, w_state, 0.5, x_ap, AOT.mult, AOT.add)
    nc.vector.tensor_scalar(s_out, w_state, 1.0, None, AOT.is_ge)
    nc.vector.scalar_tensor_tensor(w_state, w_state, 1.0, s_out, AOT.min, AOT.subtract)


# ---------------------------------------------------------------- pass 1
def build_pass1():
    nc = bass.Bass()
    x = nc.declare_dram_parameter("x", [T, 96, NF], f32, isOutput=False)
    fw_hi = nc.declare_dram_parameter("fw_hi", [2, 96, 96], bf16, isOutput=False)
    fw_lo = nc.declare_dram_parameter("fw_lo", [2, 96, 96], bf16, isOutput=False)
    fh = nc.declare_dram_parameter("fh", [3, 96, 96], f32, isOutput=False)  # Fr2, Fi2, -Fi2
    idb = nc.declare_dram_parameter("idb", [96, 96], bf16, isOutput=False)
    idf = nc.declare_dram_parameter("idf", [96, 96], f32, isOutput=False)
    w1hi = nc.declare_dram_parameter("w1hi", [2, 128, 128], bf16, isOutput=False)
    w1lo = nc.declare_dram_parameter("w1lo", [2, 128, 128], bf16, isOutput=False)
    so_d = nc.declare_dram_parameter("so_d", [T, C, HW], bf16, isOutput=True)  # sd = so - se
    se_d = nc.declare_dram_parameter("se_d", [T, C, HW], bf16, isOutput=True)  # ss = so + se
    amb_d = nc.declare_dram_parameter("amb_d", [T, C, HW], f32, isOutput=True)
    apb_d = nc.declare_dram_parameter("apb_d", [T, C, HW], f32, isOutput=True)

    with tile.TileContext(nc) as tc:
        with tc.tile_pool(name="const", bufs=1) as cpool, \
             tc.tile_pool(name="state", bufs=1) as spool, \
             tc.tile_pool(name="work", bufs=2) as wpool, \
             tc.tile_pool(name="xtp", bufs=1) as xtp, \
             tc.tile_pool(name="ps", bufs=2, space="PSUM") as ps, \
             tc.tile_pool(name="pst", bufs=2, space="PSUM") as pst:

            fwr_hi = cpool.tile([96, 96], bf16); nc.sync.dma_start(fwr_hi[:], fw_hi[0])
            fwi_hi = cpool.tile([96, 96], bf16); nc.sync.dma_start(fwi_hi[:], fw_hi[1])
            fwr_lo = cpool.tile([96, 96], bf16); nc.sync.dma_start(fwr_lo[:], fw_lo[0])
            fwi_lo = cpool.tile([96, 96], bf16); nc.sync.dma_start(fwi_lo[:], fw_lo[1])
            fhr = cpool.tile([96, 96], f32); nc.sync.dma_start(fhr[:], fh[0])
            fhi = cpool.tile([96, 96], f32); nc.sync.dma_start(fhi[:], fh[1])
            fhin = cpool.tile([96, 96], f32); nc.sync.dma_start(fhin[:], fh[2])
            idbt = cpool.tile([96, 96], bf16); nc.sync.dma_start(idbt[:], idb[:])
            idft = cpool.tile([96, 96], f32); nc.sync.dma_start(idft[:], idf[:])
            w1t = []
            for hf in range(2):
                whi = cpool.tile([128, 128], bf16, tag=f"whi{hf}")
                nc.sync.dma_start(whi[:], w1hi[hf])
                wlo = cpool.tile([128, 128], bf16, tag=f"wlo{hf}")
                nc.sync.dma_start(wlo[:], w1lo[hf])
                w1t.append((whi, wlo))

            w1s = spool.tile([NP, NF], f32); nc.vector.memset(w1s[:], 0.0)
            wr = spool.tile([NP, NF], f32); nc.vector.memset(wr[:], 0.0)
            wi = spool.tile([NP, NF], f32); nc.vector.memset(wi[:], 0.0)

            for t in range(T):
                # ---- load x[t] in spatial layout [(d,h), (c',w)]; one DMA per d
                xt = xtp.tile([NP, NF], f32, tag="xt")
                nc.sync.dma_start(xt[:], x[t])

                # ---- fused chunk loop: LIF1 -> T1 -> W1 -> T2 -> H2 -> LIF2 -> store
                off = 0
                for g in range(13):
                    cw = NCHUNK[g]
                    npairs = cw // 96
                    sl = slice(off, off + cw)
                    # LIF1 chunk
                    s_c = wpool.tile([96, 480], bf16, tag="s_c")
                    nsc0 = wpool.tile([96, 480], f32, tag="nsc0")
                    _lif_step(nc, w1s[:, sl], xt[:, sl], s_c[:, :cw], nsc0[:, :cw])
                    # W1 (contract w, bf16 hi/lo) directly on spikes
                    pr_ = ps.tile([96, 480], f32, tag="mm0")
                    nc.tensor.matmul(pr_[:, :cw], fwr_hi[:], s_c[:, :cw], start=True, stop=False)
                    nc.tensor.matmul(pr_[:, :cw], fwr_lo[:], s_c[:, :cw], start=False, stop=True)
                    pi_ = ps.tile([96, 480], f32, tag="mm1")
                    nc.tensor.matmul(pi_[:, :cw], fwi_hi[:], s_c[:, :cw], start=True, stop=False)
                    nc.tensor.matmul(pi_[:, :cw], fwi_lo[:], s_c[:, :cw], start=False, stop=True)
                    xw_r = wpool.tile([96, 480], f32, tag="xw_r")
                    xw_i = wpool.tile([96, 480], f32, tag="xw_i")
                    nc.scalar.copy(xw_r[:, :cw], pr_[:, :cw])
                    nc.vector.tensor_copy(xw_i[:, :cw], pi_[:, :cw])
                    # T2 back to [(d,h), ...] fp32, batched into one psum tile per tensor
                    pt2r = pst.tile([96, 480], f32, tag="tp")
                    for j in range(npairs):
                        js = slice(j * 96, (j + 1) * 96)
                        nc.tensor.transpose(pt2r[:, js], xw_r[:, js], idft[:])
                    xwtr = wpool.tile([96, 480], f32, tag="xwtr")
                    nc.scalar.copy(xwtr[:, :cw], pt2r[:, :cw])
                    pt2i = pst.tile([96, 480], f32, tag="tp")
                    for j in range(npairs):
                        js = slice(j * 96, (j + 1) * 96)
                        nc.tensor.transpose(pt2i[:, js], xw_i[:, js], idft[:])
                    xwti = wpool.tile([96, 480], f32, tag="xwti")
                    nc.vector.tensor_copy(xwti[:, :cw], pt2i[:, :cw])
                    # H2 (contract h, exact fp32) + LIF2 fused
                    pre = ps.tile([96, 480], f32, tag="mm0")
                    nc.tensor.matmul(pre[:, :cw], fhr[:], xwtr[:, :cw], start=True, stop=False)
                    nc.tensor.matmul(pre[:, :cw], fhin[:], xwti[:, :cw], start=False, stop=True)
                    pim = ps.tile([96, 480], f32, tag="mm1")
                    nc.tensor.matmul(pim[:, :cw], fhi[:], xwtr[:, :cw], start=True, stop=False)
                    nc.tensor.matmul(pim[:, :cw], fhr[:], xwti[:, :cw], start=False, stop=True)
                    so_c = wpool.tile([96, 480], bf16, tag="so_cc")
                    se_c = wpool.tile([96, 480], bf16, tag="se_cc")
                    _lif_step(nc, wr[:, sl], pre[:, :cw], so_c[:, :cw])
                    _lif_step(nc, wi[:, sl], pim[:, :cw], se_c[:, :cw])
                    sd_c = wpool.tile([96, 480], bf16, tag="sd_cc")
                    ss_c = wpool.tile([96, 480], bf16, tag="ss_cc")
                    nc.vector.tensor_tensor(sd_c[:, :cw], so_c[:, :cw], se_c[:, :cw], AOT.subtract)
                    nc.vector.tensor_tensor(ss_c[:, :cw], so_c[:, :cw], se_c[:, :cw], AOT.add)
                    # store spike-diff chunks to DRAM in [c][u][v] order
                    p0 = off // 96  # first pair index of chunk
                    for (tile_, dram) in ((sd_c, so_d), (ss_c, se_d)):
                        for d in range(2):
                            c0 = d * 128 + p0 * 2
                            dst2 = dram[t, c0:c0 + npairs * 2].rearrange(
                                "(pr ct) (u v) -> u pr ct v", ct=2, v=48)
                            src2 = tile_[d * 48:(d + 1) * 48, :cw].rearrange(
                                "u (pr ct v) -> u pr ct v", ct=2, v=48)
                            nc.sync.dma_start(dst2, src2)
                    off += cw

                # ---- einsum1 directly on sd/ss from c-layout reload
                for hf in range(2):
                    sd_r = wpool.tile([128, HW], bf16, tag="so_r", bufs=1)
                    nc.sync.dma_start(sd_r[:], so_d[t, hf * 128:(hf + 1) * 128, :])
                    ss_r = wpool.tile([128, HW], bf16, tag="se_r", bufs=1)
                    nc.sync.dma_start(ss_r[:], se_d[t, hf * 128:(hf + 1) * 128, :])
                    whi, wlo = w1t[hf]
                    off2 = 0
                    for ci, cw in enumerate(CCHUNK):
                        sl = slice(off2, off2 + cw)
                        pa = ps.tile([128, 512], f32, tag="mm0")
                        nc.tensor.matmul(pa[:, :cw], whi[:], sd_r[:, sl], start=True, stop=False)
                        nc.tensor.matmul(pa[:, :cw], wlo[:], sd_r[:, sl], start=False, stop=True)
                        pb = ps.tile([128, 512], f32, tag="mm1")
                        nc.tensor.matmul(pb[:, :cw], whi[:], ss_r[:, sl], start=True, stop=False)
                        nc.tensor.matmul(pb[:, :cw], wlo[:], ss_r[:, sl], start=False, stop=True)
                        amb = wpool.tile([128, 512], f32, tag="amb")
                        apb = wpool.tile([128, 512], f32, tag="apb")
                        nc.scalar.copy(amb[:, :cw], pa[:, :cw])
                        nc.vector.tensor_copy(apb[:, :cw], pb[:, :cw])
                        nc.sync.dma_start(amb_d[t, hf * 128:(hf + 1) * 128, sl], amb[:, :cw])
                        nc.sync.dma_start(apb_d[t, hf * 128:(hf + 1) * 128, sl], apb[:, :cw])
                        off2 += cw

    split_waits(nc)
    return nc


# ---------------------------------------------------------------- pass 2
def build_pass2():
    nc = bass.Bass()
    amb_d = nc.declare_dram_parameter("amb_d", [T, C, HW], f32, isOutput=False)
    apb_d = nc.declare_dram_parameter("apb_d", [T, C, HW], f32, isOutput=False)
    bn1 = nc.declare_dram_parameter("bn1", [2, 4, 128], f32, isOutput=False)  # half x (sclm,shm,sclp,shp) x c'
    w1b = nc.declare_dram_parameter("w1b", [2, 128, 128], bf16, isOutput=False)
    a2mb_d = nc.declare_dram_parameter("a2mb_d", [T, C, HW], bf16, isOutput=True)
    a2pb_d = nc.declare_dram_parameter("a2pb_d", [T, C, HW], bf16, isOutput=True)

    with tile.TileContext(nc) as tc:
        with tc.tile_pool(name="const", bufs=1) as cpool, \
             tc.tile_pool(name="state", bufs=1) as spool, \
             tc.tile_pool(name="work", bufs=2) as wpool, \
             tc.tile_pool(name="ps", bufs=2, space="PSUM") as ps:
            bnt = cpool.tile([128, 8], f32)
            nc.sync.dma_start(bnt[:], bn1[:].rearrange("h k p -> p (h k)"))
            nbig = cpool.tile([128, 1], f32)
            nc.vector.memset(nbig[:], -SBIG)
            w1bt = []
            for hf in range(2):
                wt_ = cpool.tile([128, 128], bf16, tag=f"w1b{hf}")
                nc.sync.dma_start(wt_[:], w1b[hf])
                w1bt.append(wt_)
            # LIF3 states per half per stream
            w3 = []
            for hf in range(2):
                row = []
                for st_ in range(2):
                    wst = spool.tile([128, HW], f32, tag=f"w3_{hf}_{st_}")
                    nc.vector.memset(wst[:], 0.0)
                    row.append(wst)
                w3.append(row)

            for t in range(T):
                for hf in range(2):
                    csl = slice(hf * 128, (hf + 1) * 128)
                    o1 = []
                    for st_, dram in ((0, amb_d), (1, apb_d)):
                        at = wpool.tile([128, HW], f32, tag=f"at{st_}")
                        nc.sync.dma_start(at[:], dram[t, csl, :])
                        kb = hf * 4 + st_ * 2
                        pre = wpool.tile([128, HW], f32, tag=f"pre{st_}")
                        nc.scalar.activation(pre[:], at[:], AFT.Identity,
                                             bias=bnt[:, kb + 1:kb + 2],
                                             scale=bnt[:, kb:kb + 1])
                        w = w3[hf][st_]
                        nc.vector.scalar_tensor_tensor(w[:], w[:], 0.5, pre[:],
                                                       AOT.mult, AOT.add)
                        sbf = wpool.tile([128, HW], bf16, tag=f"sbf{st_}")
                        nc.scalar.activation(sbf[:], w[:], AFT.Sigmoid,
                                             bias=nbig[:], scale=SBIG)
                        nc.vector.scalar_tensor_tensor(w[:], w[:], 1.0, w[:],
                                                       AOT.is_lt, AOT.mult)
                        o1.append(sbf)
                    o1d = wpool.tile([128, HW], bf16, tag="o1d")
                    nc.vector.tensor_tensor(o1d[:], o1[0][:], o1[1][:], AOT.subtract)
                    o1a = wpool.tile([128, HW], bf16, tag="o1a")
                    nc.vector.tensor_tensor(o1a[:], o1[0][:], o1[1][:], AOT.add)
                    a2m = wpool.tile([128, HW], bf16, tag="a2m")
                    a2p = wpool.tile([128, HW], bf16, tag="a2p")
                    wt_ = w1bt[hf]
                    off2 = 0
                    for ci, cw in enumerate(CCHUNK):
                        sl = slice(off2, off2 + cw)
                        pa = ps.tile([128, 512], f32, tag="mm0")
                        nc.tensor.matmul(pa[:, :cw], wt_[:], o1d[:, sl], start=True, stop=True)
                        pb = ps.tile([128, 512], f32, tag="mm1")
                        nc.tensor.matmul(pb[:, :cw], wt_[:], o1a[:, sl], start=True, stop=True)
                        nc.vector.tensor_copy(a2m[:, sl], pa[:, :cw])
                        nc.scalar.copy(a2p[:, sl], pb[:, :cw])
                        off2 += cw
                    nc.sync.dma_start(a2mb_d[t, csl, :], a2m[:])
                    nc.sync.dma_start(a2pb_d[t, csl, :], a2p[:])
    split_waits(nc)
    return nc


# ---------------------------------------------------------------- pass 3
def build_pass3():
    nc = bass.Bass()
    a2mb_d = nc.declare_dram_parameter("a2mb_d", [T, C, HW], bf16, isOutput=False)
    a2pb_d = nc.declare_dram_parameter("a2pb_d", [T, C, HW], bf16, isOutput=False)
    so_d = nc.declare_dram_parameter("so_d", [T, C, HW], bf16, isOutput=False)
    se_d = nc.declare_dram_parameter("se_d", [T, C, HW], bf16, isOutput=False)
    bn2 = nc.declare_dram_parameter("bn2", [2, 4, 128], f32, isOutput=False)
    fw_b = nc.declare_dram_parameter("fw_b", [2, 96, 96], bf16, isOutput=False)
    fh_b = nc.declare_dram_parameter("fh_b", [2, 96, 96], bf16, isOutput=False)
    idb = nc.declare_dram_parameter("idb", [96, 96], bf16, isOutput=False)
    yfft_d = nc.declare_dram_parameter("yfft_d", [T, C, HW], bf16, isOutput=True)
    y_d = nc.dram_tensor("y_d", [T, C, HW], bf16)

    with tile.TileContext(nc) as tc:
        with tc.tile_pool(name="const", bufs=1) as cpool, \
             tc.tile_pool(name="work", bufs=2) as wpool, \
             tc.tile_pool(name="rel", bufs=2) as rel, \
             tc.tile_pool(name="big", bufs=2) as bpool, \
             tc.tile_pool(name="psW", bufs=2, space="PSUM") as psW, \
             tc.tile_pool(name="psH", bufs=2, space="PSUM") as psH, \
             tc.tile_pool(name="pst", bufs=2, space="PSUM") as pst:
            fwr = cpool.tile([96, 96], bf16); nc.sync.dma_start(fwr[:], fw_b[0])
            fwi = cpool.tile([96, 96], bf16); nc.sync.dma_start(fwi[:], fw_b[1])
            fhr = cpool.tile([96, 96], bf16); nc.sync.dma_start(fhr[:], fh_b[0])
            fhin = cpool.tile([96, 96], bf16); nc.sync.dma_start(fhin[:], fh_b[1])
            idbt = cpool.tile([96, 96], bf16); nc.sync.dma_start(idbt[:], idb[:])
            bnt = cpool.tile([128, 8], f32)
            nc.sync.dma_start(bnt[:], bn2[:].rearrange("h k p -> p (h k)"))
            nlam = cpool.tile([128, 1], f32)
            nc.vector.memset(nlam[:], -LAM)

            for t in range(T):
                # ---- stage A (c-layout): BN2 + combine + softshrink -> y_d
                for hf in range(2):
                    csl = slice(hf * 128, (hf + 1) * 128)
                    ambc = rel.tile([128, HW], bf16, tag="ambc")
                    nc.sync.dma_start(ambc[:], a2mb_d[t, csl, :])
                    apbc = rel.tile([128, HW], bf16, tag="apbc")
                    nc.sync.dma_start(apbc[:], a2pb_d[t, csl, :])
                    so_c = rel.tile([128, HW], bf16, tag="so_c")
                    nc.sync.dma_start(so_c[:], so_d[t, csl, :])
                    se_c = rel.tile([128, HW], bf16, tag="se_c")
                    nc.sync.dma_start(se_c[:], se_d[t, csl, :])
                    kb = hf * 4
                    nc.scalar.activation(ambc[:], ambc[:], AFT.Identity,
                                         bias=bnt[:, kb + 1:kb + 2], scale=bnt[:, kb:kb + 1])
                    nc.scalar.activation(apbc[:], apbc[:], AFT.Identity,
                                         bias=bnt[:, kb + 3:kb + 4], scale=bnt[:, kb + 2:kb + 3])
                    sod = rel.tile([128, HW], bf16, tag="sod")
                    nc.vector.tensor_tensor(sod[:], so_c[:], ambc[:], AOT.mult)
                    soa = rel.tile([128, HW], bf16, tag="soa")
                    nc.vector.tensor_tensor(soa[:], se_c[:], apbc[:], AOT.mult)
                    yc = rel.tile([128, HW], bf16, tag="yc")
                    nc.vector.tensor_tensor(yc[:], sod[:], soa[:], AOT.subtract)
                    r1 = rel.tile([128, HW], bf16, tag="r1")
                    nc.scalar.activation(r1[:], yc[:], AFT.Relu, bias=nlam[:], scale=1.0)
                    r2 = rel.tile([128, HW], bf16, tag="r2")
                    nc.scalar.activation(r2[:], yc[:], AFT.Relu, bias=nlam[:], scale=-1.0)
                    nc.vector.tensor_tensor(yc[:], r1[:], r2[:], AOT.subtract)
                    nc.sync.dma_start(y_d[t, csl, :], yc[:])

                # ---- stage B: spatial load + FFT3 (W-first, bf16, real out)
                yt = bpool.tile([NP, NF], bf16, tag="yt")
                for d in range(2):
                    src = y_d[t, d * 128:(d + 1) * 128].rearrange(
                        "cp (u v) -> u cp v", v=48)
                    dst = yt[d * 48:(d + 1) * 48, :].rearrange(
                        "u (cp v) -> u cp v", v=48)
                    nc.sync.dma_start(dst, src)
                yf_sp = bpool.tile([NP, NF], bf16, tag="yf_sp")

                off = 0
                for g in range(13):
                    cw = NCHUNK[g]
                    npairs = cw // 96
                    sl = slice(off, off + cw)
                    pt1 = pst.tile([96, 480], bf16, tag="tp")
                    for j in range(npairs):
                        js = slice(off + j * 96, off + (j + 1) * 96)
                        nc.tensor.transpose(pt1[:, j * 96:(j + 1) * 96], yt[:, js], idbt[:])
                    yT = wpool.tile([96, 480], bf16, tag="yT")
                    nc.scalar.copy(yT[:, :cw], pt1[:, :cw])
                    pw_re = psW.tile([96, 480], f32, tag="mm0")
                    nc.tensor.matmul(pw_re[:, :cw], fwr[:], yT[:, :cw], start=True, stop=True)
                    pw_im = psW.tile([96, 480], f32, tag="mm1")
                    nc.tensor.matmul(pw_im[:, :cw], fwi[:], yT[:, :cw], start=True, stop=True)
                    yw_r = wpool.tile([96, 480], bf16, tag="yw_r")
                    nc.scalar.copy(yw_r[:, :cw], pw_re[:, :cw])
                    yw_i = wpool.tile([96, 480], bf16, tag="yw_i")
                    nc.vector.tensor_copy(yw_i[:, :cw], pw_im[:, :cw])
                    pt2r = pst.tile([96, 480], bf16, tag="tp")
                    for j in range(npairs):
                        js = slice(j * 96, (j + 1) * 96)
                        nc.tensor.transpose(pt2r[:, js], yw_r[:, js], idbt[:])
                    ywtr = wpool.tile([96, 480], bf16, tag="ywtr")
                    nc.vector.tensor_copy(ywtr[:, :cw], pt2r[:, :cw])
                    pt2i = pst.tile([96, 480], bf16, tag="tp")
                    for j in range(npairs):
                        js = slice(j * 96, (j + 1) * 96)
                        nc.tensor.transpose(pt2i[:, js], yw_i[:, js], idbt[:])
                    ywti = wpool.tile([96, 480], bf16, tag="ywti")
                    nc.vector.tensor_copy(ywti[:, :cw], pt2i[:, :cw])
                    ph_re = psH.tile([96, 480], f32, tag="mmh")
                    nc.tensor.matmul(ph_re[:, :cw], fhr[:], ywtr[:, :cw], start=True, stop=False)
                    nc.tensor.matmul(ph_re[:, :cw], fhin[:], ywti[:, :cw], start=False, stop=True)
                    nc.scalar.copy(yf_sp[:, sl], ph_re[:, :cw])
                    off += cw

                for d in range(2):
                    dst = yfft_d[t, d * 128:(d + 1) * 128].rearrange(
                        "(pr ct) (u v) -> u pr ct v", ct=2, v=48)
                    src = yf_sp[d * 48:(d + 1) * 48, :].rearrange(
                        "u (pr ct v) -> u pr ct v", ct=2, v=48)
                    nc.sync.dma_start(dst, src)
    split_waits(nc)
    return nc


# ---------------------------------------------------------------- pass 4
def build_pass4():
    nc = bass.Bass()
    yfft_d = nc.declare_dram_parameter("yfft_d", [T, C, HW], bf16, isOutput=False)
    bn3 = nc.declare_dram_parameter("bn3", [2, 2, 128], f32, isOutput=False)
    out_d = nc.declare_dram_parameter("out_d", [T, C, HW], f32, isOutput=True)
    with tile.TileContext(nc) as tc:
        with tc.tile_pool(name="const", bufs=1) as cpool, \
             tc.tile_pool(name="work", bufs=3) as wpool:
            bnt = cpool.tile([128, 4], f32)
            nc.sync.dma_start(bnt[:], bn3[:].rearrange("h k p -> p (h k)"))
            for t in range(T):
                for hf in range(2):
                    csl = slice(hf * 128, (hf + 1) * 128)
                    yc = wpool.tile([128, HW], bf16, tag="yc")
                    nc.sync.dma_start(yc[:], yfft_d[t, csl, :])
                    ot = wpool.tile([128, HW], f32, tag="ot")
                    nc.scalar.activation(ot[:], yc[:], AFT.Identity,
                                         bias=bnt[:, hf * 2 + 1:hf * 2 + 2],
                                         scale=bnt[:, hf * 2:hf * 2 + 1])
                    nc.sync.dma_start(out_d[t, csl, :], ot[:])
    split_waits(nc)
    return nc


# ---------------------------------------------------------------- host glue
_NCS = {}
LAST_EXEC_NS = []
LAST_PASS_NAMES = []
LAST_PROFILES = []


def _run(name, nc, in_maps, cores):
    r = run_bass_kernel_spmd(nc, in_maps, core_ids=cores)
    LAST_PASS_NAMES.append(name)
    LAST_EXEC_NS.append(r.exec_time_ns)
    LAST_PROFILES.append(r.profile_json)
    return r.results


def _get_nc(name):
    if name not in _NCS:
        _NCS[name] = {"p1": build_pass1, "p2": build_pass2,
                      "p3": build_pass3, "p4": build_pass4}[name]()
    return _NCS[name]


def _bn_affine(sums, ssqs, gamma, beta, n):
    mu = sums / n
    var = ssqs / n - mu * mu
    scl = gamma / np.sqrt(var + EPS)
    sh = beta - mu * scl
    return scl.astype(np.float32), sh.astype(np.float32)


def kernel(x, w1, g_bn1_1, b_bn1_1, g_bn1_2, b_bn1_2, g_bn2_1, b_bn2_1,
           g_bn2_2, b_bn2_2, g_bn3, b_bn3, alpha=None):
    x = np.asarray(x, np.float32)
    w1 = np.asarray(w1, np.float32)
    gb = {k: np.asarray(v, np.float32) for k, v in
          dict(g11=g_bn1_1, b11=b_bn1_1, g12=g_bn1_2, b12=b_bn1_2,
               g21=g_bn2_1, b21=b_bn2_1, g22=g_bn2_2, b22=b_bn2_2,
               g3=g_bn3, b3=b_bn3).items()}
    cores = list(range(NCORES))
    n_batch = float(T * B * HW)
    LAST_EXEC_NS.clear(); LAST_PASS_NAMES.clear(); LAST_PROFILES.clear()

    fr, fi = _dft()
    fwr_hi, fwr_lo = _hilo(_diag2(fr))
    fwi_hi, fwi_lo = _hilo(_diag2(fi))
    fh_ = np.stack([_diag2(fr), _diag2(fi), _diag2(-fi)])
    idb_ = np.eye(96).astype(_BF)
    idf_ = np.eye(96, dtype=np.float32)
    w1d = np.zeros((2, 128, 128), np.float32)
    for hf in range(2):
        for kk in range(4):
            w1d[hf, kk * 32:(kk + 1) * 32, kk * 32:(kk + 1) * 32] = w1[hf * 4 + kk]
    w1b_ = w1d.astype(_BF)
    w1hi, w1lo = _hilo(w1d)

    # ---- pass 1
    in1 = []
    for b in cores:
        in1.append({
            "x": np.ascontiguousarray(
                x[:, b].reshape(T, 2, 64, 2, H, W).transpose(0, 3, 5, 2, 1, 4)
                .reshape(T, 96, NF)),
            "fw_hi": np.stack([fwr_hi, fwi_hi]), "fw_lo": np.stack([fwr_lo, fwi_lo]),
            "fh": fh_, "idb": idb_, "idf": idf_, "w1hi": w1hi, "w1lo": w1lo,
        })
    r1 = _run("p1", _get_nc("p1"), in1, cores)

    sum_m = sum(r["amb_d"].sum(axis=(0, 2), dtype=np.float64) for r in r1)
    sum_p = sum(r["apb_d"].sum(axis=(0, 2), dtype=np.float64) for r in r1)
    ssq_m = sum(np.einsum('tcs,tcs->c', r["amb_d"], r["amb_d"],
                          dtype=np.float64) for r in r1)
    ssq_p = sum(np.einsum('tcs,tcs->c', r["apb_d"], r["apb_d"],
                          dtype=np.float64) for r in r1)
    sclm, shm = _bn_affine(sum_m, ssq_m, gb["g11"], gb["b11"], n_batch)
    sclp, shp = _bn_affine(sum_p, ssq_p, gb["g12"], gb["b12"], n_batch)
    bn1_ = np.stack([np.stack([sclm.reshape(2, 128)[h], shm.reshape(2, 128)[h],
                               sclp.reshape(2, 128)[h], shp.reshape(2, 128)[h]])
                     for h in range(2)])

    # ---- pass 2
    in2 = [{"amb_d": r1[b]["amb_d"], "apb_d": r1[b]["apb_d"], "bn1": bn1_,
            "w1b": w1d.astype(_BF)} for b in cores]
    r2 = _run("p2", _get_nc("p2"), in2, cores)

    sum_m = sum(r["a2mb_d"].astype(np.float32).sum(axis=(0, 2), dtype=np.float64)
                for r in r2)
    sum_p = sum(r["a2pb_d"].astype(np.float32).sum(axis=(0, 2), dtype=np.float64)
                for r in r2)
    ssq_m = sum(np.einsum('tcs,tcs->c', r["a2mb_d"].astype(np.float32),
                          r["a2mb_d"].astype(np.float32), dtype=np.float64) for r in r2)
    ssq_p = sum(np.einsum('tcs,tcs->c', r["a2pb_d"].astype(np.float32),
                          r["a2pb_d"].astype(np.float32), dtype=np.float64) for r in r2)
    sclm2, shm2 = _bn_affine(sum_m, ssq_m, gb["g21"], gb["b21"], n_batch)
    sclp2, shp2 = _bn_affine(sum_p, ssq_p, gb["g22"], gb["b22"], n_batch)

    bn2_ = np.stack([np.stack([sclm2.reshape(2, 128)[h], shm2.reshape(2, 128)[h],
                               sclp2.reshape(2, 128)[h], shp2.reshape(2, 128)[h]])
                     for h in range(2)])

    fw_b = np.stack([_diag2(fr), _diag2(fi)]).astype(_BF)
    fh_b = np.stack([_diag2(fr), _diag2(-fi)]).astype(_BF)

    # ---- pass 3
    in3 = [{"a2mb_d": r2[b]["a2mb_d"], "a2pb_d": r2[b]["a2pb_d"],
            "so_d": r1[b]["so_d"], "se_d": r1[b]["se_d"], "bn2": bn2_,
            "fw_b": fw_b, "fh_b": fh_b, "idb": idb_} for b in cores]
    r3 = _run("p3", _get_nc("p3"), in3, cores)

    sum3 = sum(r["yfft_d"].astype(np.float32).sum(axis=(0, 2), dtype=np.float64)
               for r in r3)
    ssq3 = sum(np.einsum('tcs,tcs->c', r["yfft_d"].astype(np.float32),
                         r["yfft_d"].astype(np.float32), dtype=np.float64) for r in r3)
    scl3, sh3 = _bn_affine(sum3, ssq3, gb["g3"], gb["b3"], n_batch)
    bn3_ = np.stack([np.stack([scl3.reshape(2, 128)[h], sh3.reshape(2, 128)[h]])
                     for h in range(2)])

    # ---- pass 4
    in4 = [{"yfft_d": r3[b]["yfft_d"], "bn3": bn3_} for b in cores]
    r4 = _run("p4", _get_nc("p4"), in4, cores)

    out = np.empty((T, B, C, H, W), np.float32)
    for b in cores:
        out[:, b] = r4[b]["out_d"].reshape(T, C, H, W)
    return out



# revision 15
# speedup vs baseline: 1.0274x; 1.0274x over previous
"""Trainium2 Bass kernel for the spiking spectral net (nn_ASFF).

Pipeline: LIF -> FFT2 -> LIF -> blockdiag matmul -> BN -> LIF -> blockdiag
matmul -> BN -> combine -> softshrink -> FFT2.real -> BN.

Sharding: data-parallel over B (8 samples -> 8 cores). Four SPMD NEFF passes
with host-side all-reduce of BatchNorm statistics between them (stats are
[C]-vectors; everything heavy stays on device).

Layout notes:
 - c-layout: [128 partitions = half of C, 2304 free = (u,v) flattened hw]
 - spatial layout: [96 partitions = (d,h) with d = C-half, 6144 free]
 - FFT2 per 48x48 tile is done as W-side DFT (contract w), TensorE
   transpose, H-side DFT (contract h). DFT matrices are symmetric.
 - Matmuls on spike inputs use bf16 hi/lo-split DFT/weight matrices
   (exact to ~2^-17); the second FFT side has continuous input and uses
   exact fp32 matmuls. Post-threshold math (pass 3) is bf16 throughout.
"""
import sys
sys.path.insert(0, '/opt/trn_rl_repo')
import numpy as np
import ml_dtypes
import concourse.bass as bass
import concourse.tile as tile
import concourse.mybir as mybir
from concourse.bass_utils import run_bass_kernel_spmd

f32, bf16, f32r = mybir.dt.float32, mybir.dt.bfloat16, mybir.dt.float32r
AOT = mybir.AluOpType
AFT = mybir.ActivationFunctionType

T, B, C, H, W = 4, 8, 256, 48, 48
K, BS = 8, 32
HW = H * W            # 2304
NCORES = 8
NP = 96               # spatial-layout partitions (2 c-halves x 48)
NF = 6144             # spatial-layout free size (64 pairs x 2 x 48)
LAM = 0.06
EPS = 1e-5
NCHUNK = [480] * 12 + [384]           # spatial free chunking (5/4 pairs each)
CCHUNK = [512, 512, 512, 512, 256]    # c-layout free chunking of 2304

_BF = ml_dtypes.bfloat16
SBIG = float(2 ** 30)


def _hilo(x):
    hi = x.astype(np.float32).astype(_BF)
    lo = (x.astype(np.float32) - hi.astype(np.float32)).astype(_BF)
    return hi, lo


def _dft():
    j = np.arange(48)
    ang = -2.0 * np.pi * np.outer(j, j) / 48.0
    fr = (np.cos(ang) / np.sqrt(48.0)).astype(np.float32)
    fi = (np.sin(ang) / np.sqrt(48.0)).astype(np.float32)
    return fr, fi


def _diag2(m):
    out = np.zeros((96, 96), m.dtype)
    out[:48, :48] = m
    out[48:, 48:] = m
    return out


def split_waits(nc, max_waits=1):
    """This toolchain's walrus only tolerates one sync-wait per instruction;
    spill extra waits onto NoOps inserted just before the instruction."""
    ctr = 0
    for f in nc.m.functions:
        for bb in f.blocks:
            insts = list(bb.instructions)
            out = []
            changed = False
            for inst in insts:
                si = inst.sync_info
                waits = list(si.on_wait) if si else []
                if len(waits) > max_waits:
                    for wcond in waits[:-max_waits]:
                        ctr += 1
                        nop = mybir.InstNoOp(name=f"wsplit-{ctr}")
                        nop.engine = inst.engine
                        nop.sync_info = mybir.SyncInfo(on_wait=[wcond], on_update=[])
                        out.append(nop)
                    si.on_wait = waits[-max_waits:]
                    changed = True
                out.append(inst)
            if changed:
                bb.instructions = out
    return ctr


def _lif_step(nc, w_state, x_ap, s_out, ns_scratch=None):
    """One LIF step on w = 2*v scaled state: u = 0.5*w + x (into w_state),
    s = (u >= 1), w = min(u,1) - s  (== u*(u<1) bitwise).  x_ap may be PSUM."""
    nc.vector.scalar_tensor_tensor(w_state, w_state, 0.5, x_ap, AOT.mult, AOT.add)
    nc.vector.tensor_scalar(s_out, w_state, 1.0, None, AOT.is_ge)
    nc.vector.scalar_tensor_tensor(w_state, w_state, 1.0, s_out, AOT.min, AOT.subtract)


# ---------------------------------------------------------------- pass 1
def build_pass1():
    nc = bass.Bass()
    x = nc.declare_dram_parameter("x", [T, 96, NF], f32, isOutput=False)
    fw_hi = nc.declare_dram_parameter("fw_hi", [2, 96, 96], bf16, isOutput=False)
    fw_lo = nc.declare_dram_parameter("fw_lo", [2, 96, 96], bf16, isOutput=False)
    fh = nc.declare_dram_parameter("fh", [3, 96, 96], f32, isOutput=False)  # Fr2, Fi2, -Fi2
    idb = nc.declare_dram_parameter("idb", [96, 96], bf16, isOutput=False)
    idf = nc.declare_dram_parameter("idf", [96, 96], f32, isOutput=False)
    w1hi = nc.declare_dram_parameter("w1hi", [2, 128, 128], bf16, isOutput=False)
    w1lo = nc.declare_dram_parameter("w1lo", [2, 128, 128], bf16, isOutput=False)
    so_d = nc.declare_dram_parameter("so_d", [T, C, HW], bf16, isOutput=True)  # sd = so - se
    se_d = nc.declare_dram_parameter("se_d", [T, C, HW], bf16, isOutput=True)  # ss = so + se
    amb_d = nc.declare_dram_parameter("amb_d", [T, C, HW], f32, isOutput=True)
    apb_d = nc.declare_dram_parameter("apb_d", [T, C, HW], f32, isOutput=True)

    with tile.TileContext(nc) as tc:
        with tc.tile_pool(name="const", bufs=1) as cpool, \
             tc.tile_pool(name="state", bufs=1) as spool, \
             tc.tile_pool(name="work", bufs=2) as wpool, \
             tc.tile_pool(name="xtp", bufs=1) as xtp, \
             tc.tile_pool(name="ps", bufs=2, space="PSUM") as ps, \
             tc.tile_pool(name="pst", bufs=2, space="PSUM") as pst:

            fwr_hi = cpool.tile([96, 96], bf16); nc.sync.dma_start(fwr_hi[:], fw_hi[0])
            fwi_hi = cpool.tile([96, 96], bf16); nc.sync.dma_start(fwi_hi[:], fw_hi[1])
            fwr_lo = cpool.tile([96, 96], bf16); nc.sync.dma_start(fwr_lo[:], fw_lo[0])
            fwi_lo = cpool.tile([96, 96], bf16); nc.sync.dma_start(fwi_lo[:], fw_lo[1])
            fhr = cpool.tile([96, 96], f32); nc.sync.dma_start(fhr[:], fh[0])
            fhi = cpool.tile([96, 96], f32); nc.sync.dma_start(fhi[:], fh[1])
            fhin = cpool.tile([96, 96], f32); nc.sync.dma_start(fhin[:], fh[2])
            idbt = cpool.tile([96, 96], bf16); nc.sync.dma_start(idbt[:], idb[:])
            idft = cpool.tile([96, 96], f32); nc.sync.dma_start(idft[:], idf[:])
            w1t = []
            for hf in range(2):
                whi = cpool.tile([128, 128], bf16, tag=f"whi{hf}")
                nc.sync.dma_start(whi[:], w1hi[hf])
                wlo = cpool.tile([128, 128], bf16, tag=f"wlo{hf}")
                nc.sync.dma_start(wlo[:], w1lo[hf])
                w1t.append((whi, wlo))

            w1s = spool.tile([NP, NF], f32); nc.vector.memset(w1s[:], 0.0)
            wr = spool.tile([NP, NF], f32); nc.vector.memset(wr[:], 0.0)
            wi = spool.tile([NP, NF], f32); nc.vector.memset(wi[:], 0.0)

            for t in range(T):
                # ---- load x[t] in spatial layout [(d,h), (c',w)]; one DMA per d
                xt = xtp.tile([NP, NF], f32, tag="xt")
                nc.sync.dma_start(xt[:], x[t])

                # ---- fused chunk loop: LIF1 -> T1 -> W1 -> T2 -> H2 -> LIF2 -> store
                off = 0
                for g in range(13):
                    cw = NCHUNK[g]
                    npairs = cw // 96
                    sl = slice(off, off + cw)
                    # LIF1 chunk
                    s_c = wpool.tile([96, 480], bf16, tag="s_c")
                    nsc0 = wpool.tile([96, 480], f32, tag="nsc0")
                    _lif_step(nc, w1s[:, sl], xt[:, sl], s_c[:, :cw], nsc0[:, :cw])
                    # W1 (contract w, bf16 hi/lo) directly on spikes
                    pr_ = ps.tile([96, 480], f32, tag="mm0")
                    nc.tensor.matmul(pr_[:, :cw], fwr_hi[:], s_c[:, :cw], start=True, stop=False)
                    nc.tensor.matmul(pr_[:, :cw], fwr_lo[:], s_c[:, :cw], start=False, stop=True)
                    pi_ = ps.tile([96, 480], f32, tag="mm1")
                    nc.tensor.matmul(pi_[:, :cw], fwi_hi[:], s_c[:, :cw], start=True, stop=False)
                    nc.tensor.matmul(pi_[:, :cw], fwi_lo[:], s_c[:, :cw], start=False, stop=True)
                    xw_r = wpool.tile([96, 480], f32, tag="xw_r")
                    xw_i = wpool.tile([96, 480], f32, tag="xw_i")
                    nc.scalar.copy(xw_r[:, :cw], pr_[:, :cw])
                    nc.vector.tensor_copy(xw_i[:, :cw], pi_[:, :cw])
                    # T2 back to [(d,h), ...] fp32, batched into one psum tile per tensor
                    pt2r = pst.tile([96, 480], f32, tag="tp")
                    for j in range(npairs):
                        js = slice(j * 96, (j + 1) * 96)
                        nc.tensor.transpose(pt2r[:, js], xw_r[:, js], idft[:])
                    xwtr = wpool.tile([96, 480], f32, tag="xwtr")
                    nc.scalar.copy(xwtr[:, :cw], pt2r[:, :cw])
                    pt2i = pst.tile([96, 480], f32, tag="tp")
                    for j in range(npairs):
                        js = slice(j * 96, (j + 1) * 96)
                        nc.tensor.transpose(pt2i[:, js], xw_i[:, js], idft[:])
                    xwti = wpool.tile([96, 480], f32, tag="xwti")
                    nc.vector.tensor_copy(xwti[:, :cw], pt2i[:, :cw])
                    # H2 (contract h, exact fp32) + LIF2 fused
                    pre = ps.tile([96, 480], f32, tag="mm0")
                    nc.tensor.matmul(pre[:, :cw], fhr[:], xwtr[:, :cw], start=True, stop=False)
                    nc.tensor.matmul(pre[:, :cw], fhin[:], xwti[:, :cw], start=False, stop=True)
                    pim = ps.tile([96, 480], f32, tag="mm1")
                    nc.tensor.matmul(pim[:, :cw], fhi[:], xwtr[:, :cw], start=True, stop=False)
                    nc.tensor.matmul(pim[:, :cw], fhr[:], xwti[:, :cw], start=False, stop=True)
                    so_c = wpool.tile([96, 480], bf16, tag="so_cc")
                    se_c = wpool.tile([96, 480], bf16, tag="se_cc")
                    _lif_step(nc, wr[:, sl], pre[:, :cw], so_c[:, :cw])
                    _lif_step(nc, wi[:, sl], pim[:, :cw], se_c[:, :cw])
                    sd_c = wpool.tile([96, 480], bf16, tag="sd_cc")
                    ss_c = wpool.tile([96, 480], bf16, tag="ss_cc")
                    nc.vector.tensor_tensor(sd_c[:, :cw], so_c[:, :cw], se_c[:, :cw], AOT.subtract)
                    nc.vector.tensor_tensor(ss_c[:, :cw], so_c[:, :cw], se_c[:, :cw], AOT.add)
                    # store spike-diff chunks to DRAM in [c][u][v] order
                    p0 = off // 96  # first pair index of chunk
                    for (tile_, dram) in ((sd_c, so_d), (ss_c, se_d)):
                        for d in range(2):
                            c0 = d * 128 + p0 * 2
                            dst2 = dram[t, c0:c0 + npairs * 2].rearrange(
                                "(pr ct) (u v) -> u pr ct v", ct=2, v=48)
                            src2 = tile_[d * 48:(d + 1) * 48, :cw].rearrange(
                                "u (pr ct v) -> u pr ct v", ct=2, v=48)
                            nc.sync.dma_start(dst2, src2)
                    off += cw

                # ---- einsum1 directly on sd/ss from c-layout reload
                for hf in range(2):
                    sd_r = wpool.tile([128, HW], bf16, tag="so_r", bufs=1)
                    nc.sync.dma_start(sd_r[:], so_d[t, hf * 128:(hf + 1) * 128, :])
                    ss_r = wpool.tile([128, HW], bf16, tag="se_r", bufs=1)
                    nc.sync.dma_start(ss_r[:], se_d[t, hf * 128:(hf + 1) * 128, :])
                    whi, wlo = w1t[hf]
                    off2 = 0
                    for ci, cw in enumerate(CCHUNK):
                        sl = slice(off2, off2 + cw)
                        pa = ps.tile([128, 512], f32, tag="mm0")
                        nc.tensor.matmul(pa[:, :cw], whi[:], sd_r[:, sl], start=True, stop=False)
                        nc.tensor.matmul(pa[:, :cw], wlo[:], sd_r[:, sl], start=False, stop=True)
                        pb = ps.tile([128, 512], f32, tag="mm1")
                        nc.tensor.matmul(pb[:, :cw], whi[:], ss_r[:, sl], start=True, stop=False)
                        nc.tensor.matmul(pb[:, :cw], wlo[:], ss_r[:, sl], start=False, stop=True)
                        amb = wpool.tile([128, 512], f32, tag="amb")
                        apb = wpool.tile([128, 512], f32, tag="apb")
                        nc.scalar.copy(amb[:, :cw], pa[:, :cw])
                        nc.vector.tensor_copy(apb[:, :cw], pb[:, :cw])
                        nc.sync.dma_start(amb_d[t, hf * 128:(hf + 1) * 128, sl], amb[:, :cw])
                        nc.sync.dma_start(apb_d[t, hf * 128:(hf + 1) * 128, sl], apb[:, :cw])
                        off2 += cw

    split_waits(nc)
    return nc


# ------------------------------------------------------- fused pass B (2+3+4)
# Channel-block sharded: core k owns channels [32k, 32k+32) for ALL samples.
# c-layout tiles: [128 = (4 samples x 32 ch), 2304 free].  Free order is (v,u)
# for everything before FFT3 so the spatial reload needs no pre-transpose.
NB = T * B * HW * 1.0   # per-channel count for BN stats


def build_passB():
    nc = bass.Bass()
    amb_B = nc.declare_dram_parameter("amb_B", [T, 2, 128, HW], f32, isOutput=False)
    apb_B = nc.declare_dram_parameter("apb_B", [T, 2, 128, HW], f32, isOutput=False)
    sd_B = nc.declare_dram_parameter("sd_B", [T, 2, 128, HW], bf16, isOutput=False)
    ss_B = nc.declare_dram_parameter("ss_B", [T, 2, 128, HW], bf16, isOutput=False)
    bn1 = nc.declare_dram_parameter("bn1", [4, 128], f32, isOutput=False)  # sclm,shm,sclp,shp
    w1d4 = nc.declare_dram_parameter("w1d4", [128, 128], bf16, isOutput=False)
    fwb = nc.declare_dram_parameter("fwb", [2, 96, 96], bf16, isOutput=False)
    fhb = nc.declare_dram_parameter("fhb", [2, 96, 96], bf16, isOutput=False)
    idb = nc.declare_dram_parameter("idb", [96, 96], bf16, isOutput=False)
    gb = nc.declare_dram_parameter("gb", [6, 32], f32, isOutput=False)  # g21,b21,g22,b22,g3,b3
    ssel = nc.declare_dram_parameter("ssel", [128, 32], f32, isOutput=False)
    out_B = nc.declare_dram_parameter("out_B", [T, 2, 128, HW], f32, isOutput=True)
    y_d = nc.dram_tensor("y_d", [T, B, 32, 48, 48], bf16)       # (v,u) free order
    a2m_d = nc.dram_tensor("a2m_d", [T, 2, 128, HW], bf16)
    a2p_d = nc.dram_tensor("a2p_d", [T, 2, 128, HW], bf16)
    yf_d = nc.dram_tensor("yf_d", [T, B, 32, 48, 48], bf16)     # (u,v) free order
    invN = 1.0 / NB

    def bn_finalize(wpool, stats4, gcol, scl128s, sh128s, n_streams=2):
        """stats4: [32,4] sbuf cols (sum_m, ssq_m, sum_p, ssq_p) already folded.
        Writes per-partition [128,1] scl/sh tiles per stream."""
        mean4 = wpool.tile([32, 4], f32, tag="mean4")
        nc.vector.tensor_scalar(mean4[:], stats4[:], invN, None, AOT.mult)
        for st_ in range(n_streams):
            mcol = mean4[:, 2 * st_:2 * st_ + 1]
            ecol = mean4[:, 2 * st_ + 1:2 * st_ + 2]
            m2 = wpool.tile([32, 1], f32, tag=f"m2_{st_}")
            nc.vector.tensor_tensor(m2[:], mcol, mcol, AOT.mult)
            var = wpool.tile([32, 1], f32, tag=f"var_{st_}")
            nc.vector.tensor_tensor(var[:], ecol, m2[:], AOT.subtract)
            nc.vector.tensor_scalar(var[:], var[:], EPS, None, AOT.add)
            std = wpool.tile([32, 1], f32, tag=f"std_{st_}")
            nc.scalar.activation(std[:], var[:], AFT.Sqrt)
            rstd = wpool.tile([32, 1], f32, tag=f"rstd_{st_}")
            nc.vector.reciprocal(rstd[:], std[:])
            scl32 = wpool.tile([32, 1], f32, tag=f"scl32_{st_}")
            nc.vector.tensor_tensor(scl32[:], gcol[2 * st_], rstd[:], AOT.mult)
            tmp = wpool.tile([32, 1], f32, tag=f"shtmp_{st_}")
            nc.vector.tensor_tensor(tmp[:], mcol, scl32[:], AOT.mult)
            sh32 = wpool.tile([32, 1], f32, tag=f"sh32_{st_}")
            nc.vector.tensor_tensor(sh32[:], gcol[2 * st_ + 1], tmp[:], AOT.subtract)
            for m in range(4):
                nc.sync.dma_start(scl128s[st_][32 * m:32 * m + 32, :], scl32[:])
                nc.sync.dma_start(sh128s[st_][32 * m:32 * m + 32, :], sh32[:])

    with tile.TileContext(nc) as tc:
        with tc.tile_pool(name="const", bufs=1) as cpool, \
             tc.tile_pool(name="bnsb", bufs=1) as bpool:
            bn1t = cpool.tile([128, 4], f32)
            nc.sync.dma_start(bn1t[:], bn1[:].rearrange("k p -> p k"))
            nbig = cpool.tile([128, 1], f32); nc.vector.memset(nbig[:], -SBIG)
            nlam = cpool.tile([128, 1], f32); nc.vector.memset(nlam[:], -LAM)
            w1t = cpool.tile([128, 128], bf16); nc.sync.dma_start(w1t[:], w1d4[:])
            fwr = cpool.tile([96, 96], bf16); nc.sync.dma_start(fwr[:], fwb[0])
            fwi = cpool.tile([96, 96], bf16); nc.sync.dma_start(fwi[:], fwb[1])
            fhr = cpool.tile([96, 96], bf16); nc.sync.dma_start(fhr[:], fhb[0])
            fhin = cpool.tile([96, 96], bf16); nc.sync.dma_start(fhin[:], fhb[1])
            idbt = cpool.tile([96, 96], bf16); nc.sync.dma_start(idbt[:], idb[:])
            gbt = cpool.tile([32, 6], f32)
            nc.sync.dma_start(gbt[:], gb[:].rearrange("k c -> c k"))
            sselt = cpool.tile([128, 32], f32); nc.sync.dma_start(sselt[:], ssel[:])
            # stat collectors (col = t*2+g) and bn tiles
            coll = {}
            for nm in ("sum_m", "ssq_m", "sum_p", "ssq_p", "sum_3", "ssq_3"):
                coltile = bpool.tile([128, 8], f32, tag=f"coll_{nm}")
                coll[nm] = coltile
            scl2 = []
            sh2 = []
            for s in range(2):
                sctile = bpool.tile([128, 1], f32, tag=f"scl2_{s}")
                scl2.append(sctile)
                shtile = bpool.tile([128, 1], f32, tag=f"sh2_{s}")
                sh2.append(shtile)
            scl3t = bpool.tile([128, 1], f32, tag="scl3")
            sh3t = bpool.tile([128, 1], f32, tag="sh3")
            scl3 = [scl3t]
            sh3 = [sh3t]

            with tc.tile_pool(name="a2res", bufs=1) as a2pool:

                # ---------------- stage 2: BN1 affine -> LIF3 -> einsum2 + stats
                with tc.tile_pool(name="s2w", bufs=2) as wpool, \
                     tc.tile_pool(name="s2st", bufs=1) as spool, \
                     tc.tile_pool(name="s2ps", bufs=2, space="PSUM") as ps:
                    w3 = {}
                    for g in range(2):
                        for st_ in range(2):
                            wtile = spool.tile([128, HW], f32, tag=f"w3_{g}_{st_}")
                            w3[g, st_] = wtile
                            nc.vector.memset(wtile[:], 0.0)
                    for t in range(T):
                        for g in range(2):
                            col = t * 2 + g
                            o1 = []
                            for st_, dram in ((0, amb_B), (1, apb_B)):
                                at = wpool.tile([128, HW], f32, tag=f"at{st_}")
                                nc.sync.dma_start(at[:], dram[t, g])
                                kb = st_ * 2
                                pre = wpool.tile([128, HW], f32, tag=f"pre{st_}")
                                nc.scalar.activation(pre[:], at[:], AFT.Identity,
                                                     bias=bn1t[:, kb + 1:kb + 2],
                                                     scale=bn1t[:, kb:kb + 1])
                                w = w3[g, st_]
                                nc.vector.scalar_tensor_tensor(w[:], w[:], 0.5, pre[:],
                                                               AOT.mult, AOT.add)
                                sbf = wpool.tile([128, HW], bf16, tag=f"sbf{st_}")
                                nc.scalar.activation(sbf[:], w[:], AFT.Sigmoid,
                                                     bias=nbig[:], scale=SBIG)
                                nc.vector.scalar_tensor_tensor(w[:], w[:], 1.0, w[:],
                                                               AOT.is_lt, AOT.mult)
                                o1.append(sbf)
                            o1d = wpool.tile([128, HW], bf16, tag="o1d")
                            nc.vector.tensor_tensor(o1d[:], o1[0][:], o1[1][:], AOT.subtract)
                            o1a = wpool.tile([128, HW], bf16, tag="o1a")
                            nc.vector.tensor_tensor(o1a[:], o1[0][:], o1[1][:], AOT.add)
                            a2m = wpool.tile([128, HW], bf16, tag="a2m")
                            a2p = wpool.tile([128, HW], bf16, tag="a2p")
                            off2 = 0
                            for cw in CCHUNK:
                                sl = slice(off2, off2 + cw)
                                pa = ps.tile([128, 512], f32, tag="mm0")
                                nc.tensor.matmul(pa[:, :cw], w1t[:], o1d[:, sl], start=True, stop=True)
                                pb = ps.tile([128, 512], f32, tag="mm1")
                                nc.tensor.matmul(pb[:, :cw], w1t[:], o1a[:, sl], start=True, stop=True)
                                nc.vector.tensor_copy(a2m[:, sl], pa[:, :cw])
                                nc.scalar.copy(a2p[:, sl], pb[:, :cw])
                                off2 += cw
                            # stats for BN2 (per-partition partials into column col)
                            sqs = wpool.tile([128, HW], bf16, tag="sqs")
                            nc.scalar.activation(sqs[:], a2m[:], AFT.Square,
                                                 accum_out=coll["ssq_m"][:, col:col + 1])
                            nc.vector.tensor_reduce(coll["sum_m"][:, col:col + 1], a2m[:],
                                                    mybir.AxisListType.X, AOT.add)
                            sqs2 = wpool.tile([128, HW], bf16, tag="sqs")
                            nc.scalar.activation(sqs2[:], a2p[:], AFT.Square,
                                                 accum_out=coll["ssq_p"][:, col:col + 1])
                            nc.vector.tensor_reduce(coll["sum_p"][:, col:col + 1], a2p[:],
                                                    mybir.AxisListType.X, AOT.add)
                            nc.sync.dma_start(a2m_d[t, g], a2m[:])
                            nc.sync.dma_start(a2p_d[t, g], a2p[:])

                # ---------------- BN2 stats finalize
                with tc.tile_pool(name="bnf", bufs=1) as fpool, \
                     tc.tile_pool(name="bnfps", bufs=1, space="PSUM") as fps:
                    stats4p = fpool.tile([128, 4], f32, tag="stats4p")
                    for i, nm in enumerate(("sum_m", "ssq_m", "sum_p", "ssq_p")):
                        nc.vector.tensor_reduce(stats4p[:, i:i + 1], coll[nm][:],
                                                mybir.AxisListType.X, AOT.add)
                    pf = fps.tile([32, 4], f32, tag="fold")
                    nc.tensor.matmul(pf[:], sselt[:], stats4p[:], start=True, stop=True)
                    stats4 = fpool.tile([32, 4], f32, tag="stats4")
                    nc.vector.tensor_copy(stats4[:], pf[:])
                    gcol = [gbt[:, i:i + 1] for i in range(4)]
                    bn_finalize(fpool, stats4, gcol, scl2, sh2)

                # ---------------- stage 3: BN2 affine + combine + softshrink + FFT3
                with tc.tile_pool(name="s3w", bufs=2) as wpool, \
                     tc.tile_pool(name="s3f", bufs=2) as fw_, \
                     tc.tile_pool(name="psW", bufs=2, space="PSUM") as psW, \
                     tc.tile_pool(name="psH", bufs=2, space="PSUM") as psH, \
                     tc.tile_pool(name="pst", bufs=2, space="PSUM") as pst:
                    for t in range(T):
                        for g in range(2):
                            a2mt = wpool.tile([128, HW], bf16, tag="a2mt")
                            nc.sync.dma_start(a2mt[:], a2m_d[t, g])
                            a2pt = wpool.tile([128, HW], bf16, tag="a2pt")
                            nc.sync.dma_start(a2pt[:], a2p_d[t, g])
                            o2m = wpool.tile([128, HW], bf16, tag="o2m")
                            nc.scalar.activation(o2m[:], a2mt[:], AFT.Identity,
                                                 bias=sh2[0][:], scale=scl2[0][:])
                            o2p = wpool.tile([128, HW], bf16, tag="o2p")
                            nc.scalar.activation(o2p[:], a2pt[:], AFT.Identity,
                                                 bias=sh2[1][:], scale=scl2[1][:])
                            sdt = wpool.tile([128, HW], bf16, tag="sdt")
                            nc.sync.dma_start(sdt[:], sd_B[t, g])
                            sst = wpool.tile([128, HW], bf16, tag="sst")
                            nc.sync.dma_start(sst[:], ss_B[t, g])
                            m1 = wpool.tile([128, HW], bf16, tag="m1")
                            nc.vector.tensor_tensor(m1[:], sdt[:], o2m[:], AOT.mult)
                            m2_ = wpool.tile([128, HW], bf16, tag="m2c")
                            nc.vector.tensor_tensor(m2_[:], sst[:], o2p[:], AOT.mult)
                            yc = wpool.tile([128, HW], bf16, tag="yc")
                            nc.vector.tensor_tensor(yc[:], m1[:], m2_[:], AOT.subtract)
                            r1 = wpool.tile([128, HW], bf16, tag="r1")
                            nc.scalar.activation(r1[:], yc[:], AFT.Relu, bias=nlam[:], scale=1.0)
                            r2 = wpool.tile([128, HW], bf16, tag="r2")
                            nc.scalar.activation(r2[:], yc[:], AFT.Relu, bias=nlam[:], scale=-1.0)
                            nc.vector.tensor_tensor(yc[:], r1[:], r2[:], AOT.subtract)
                            dst = y_d[t, 4 * g:4 * g + 4].rearrange("m c v u -> (m c) (v u)")
                            nc.sync.dma_start(dst, yc[:])

                        # FFT3 per sample pair (contract v, transpose, contract u)
                        for sp in range(4):
                            yt = fw_.tile([96, 1536], bf16, tag="yt")
                            for m in range(2):
                                nc.sync.dma_start(
                                    yt[48 * m:48 * m + 48, :].rearrange(
                                        "v (c u) -> v c u", c=32),
                                    y_d[t, 2 * sp + m].rearrange("c v u -> v c u"))
                            yfs = fw_.tile([96, 1536], bf16, tag="yfs")
                            off = 0
                            for cw in (480, 480, 480, 96):
                                npairs = cw // 96
                                sl = slice(off, off + cw)
                                pw_re = psW.tile([96, 480], f32, tag="mm0")
                                nc.tensor.matmul(pw_re[:, :cw], fwr[:], yt[:, sl], start=True, stop=True)
                                pw_im = psW.tile([96, 480], f32, tag="mm1")
                                nc.tensor.matmul(pw_im[:, :cw], fwi[:], yt[:, sl], start=True, stop=True)
                                yw_r = wpool.tile([96, 480], bf16, tag="yw_r")
                                nc.scalar.copy(yw_r[:, :cw], pw_re[:, :cw])
                                yw_i = wpool.tile([96, 480], bf16, tag="yw_i")
                                nc.vector.tensor_copy(yw_i[:, :cw], pw_im[:, :cw])
                                pt2r = pst.tile([96, 480], bf16, tag="tp")
                                for j in range(npairs):
                                    js = slice(j * 96, (j + 1) * 96)
                                    nc.tensor.transpose(pt2r[:, js], yw_r[:, js], idbt[:])
                                ywtr = wpool.tile([96, 480], bf16, tag="ywtr")
                                nc.vector.tensor_copy(ywtr[:, :cw], pt2r[:, :cw])
                                pt2i = pst.tile([96, 480], bf16, tag="tp")
                                for j in range(npairs):
                                    js = slice(j * 96, (j + 1) * 96)
                                    nc.tensor.transpose(pt2i[:, js], yw_i[:, js], idbt[:])
                                ywti = wpool.tile([96, 480], bf16, tag="ywti")
                                nc.scalar.copy(ywti[:, :cw], pt2i[:, :cw])
                                ph = psH.tile([96, 480], f32, tag="mmh")
                                nc.tensor.matmul(ph[:, :cw], fhr[:], ywtr[:, :cw], start=True, stop=False)
                                nc.tensor.matmul(ph[:, :cw], fhin[:], ywti[:, :cw], start=False, stop=True)
                                nc.scalar.copy(yfs[:, sl], ph[:, :cw])
                                off += cw
                            srcf = yfs[:].rearrange("p (cp m v) -> p cp m v", cp=16, m=2)
                            for m in range(2):
                                dstf = yf_d[t, 2 * sp + m].rearrange(
                                    "(cp c2) u v -> (c2 u) cp v", c2=2)
                                nc.sync.dma_start(dstf, srcf[:, :, m, :])

            # ---------------- stage 4: BN3 stats + affine + out
            with tc.tile_pool(name="yfres", bufs=1) as yfpool, \
                 tc.tile_pool(name="s4w", bufs=2) as wpool4, \
                 tc.tile_pool(name="s4ps", bufs=1, space="PSUM") as fps4:
                yf_res = {}
                for t in range(T):
                    for g in range(2):
                        col = t * 2 + g
                        yft = yfpool.tile([128, HW], bf16, tag=f"yf_{t}_{g}")
                        yf_res[t, g] = yft
                        nc.sync.dma_start(
                            yft[:], yf_d[t, 4 * g:4 * g + 4].rearrange("m c u v -> (m c) (u v)"))
                        sq3 = wpool4.tile([128, HW], bf16, tag="sq3")
                        nc.scalar.activation(sq3[:], yft[:], AFT.Square,
                                             accum_out=coll["ssq_3"][:, col:col + 1])
                        nc.vector.tensor_reduce(coll["sum_3"][:, col:col + 1], yft[:],
                                                mybir.AxisListType.X, AOT.add)
                stats4p3 = wpool4.tile([128, 4], f32, tag="stats4p3")
                nc.vector.memset(stats4p3[:], 0.0)
                for i, nm in enumerate(("sum_3", "ssq_3")):
                    nc.vector.tensor_reduce(stats4p3[:, i:i + 1], coll[nm][:],
                                            mybir.AxisListType.X, AOT.add)
                pf3 = fps4.tile([32, 4], f32, tag="fold3")
                nc.tensor.matmul(pf3[:], sselt[:], stats4p3[:], start=True, stop=True)
                stats43 = wpool4.tile([32, 4], f32, tag="stats43")
                nc.vector.tensor_copy(stats43[:], pf3[:])
                gcol3 = [gbt[:, 4:5], gbt[:, 5:6]]
                bn_finalize(wpool4, stats43, gcol3, scl3, sh3, n_streams=1)
                for t in range(T):
                    for g in range(2):
                        ot = wpool4.tile([128, HW], f32, tag="ot")
                        nc.scalar.activation(ot[:], yf_res[t, g][:], AFT.Identity,
                                             bias=sh3[0][:], scale=scl3[0][:])
                        nc.sync.dma_start(out_B[t, g], ot[:])
    split_waits(nc)
    return nc


# ---------------------------------------------------------------- host glue
_NCS = {}
LAST_EXEC_NS = []
LAST_PASS_NAMES = []
LAST_PROFILES = []


def _run(name, nc, in_maps, cores):
    r = run_bass_kernel_spmd(nc, in_maps, core_ids=cores)
    LAST_PASS_NAMES.append(name)
    LAST_EXEC_NS.append(r.exec_time_ns)
    LAST_PROFILES.append(r.profile_json)
    return r.results


def _get_nc(name):
    if name not in _NCS:
        _NCS[name] = {"p1": build_pass1, "pB": build_passB}[name]()
    return _NCS[name]


def _bn_affine(sums, ssqs, gamma, beta, n):
    mu = sums / n
    var = ssqs / n - mu * mu
    scl = gamma / np.sqrt(var + EPS)
    sh = beta - mu * scl
    return scl.astype(np.float32), sh.astype(np.float32)


def kernel(x, w1, g_bn1_1, b_bn1_1, g_bn1_2, b_bn1_2, g_bn2_1, b_bn2_1,
           g_bn2_2, b_bn2_2, g_bn3, b_bn3, alpha=None):
    x = np.asarray(x, np.float32)
    w1 = np.asarray(w1, np.float32)
    gb_ = {k: np.asarray(v, np.float32) for k, v in
           dict(g11=g_bn1_1, b11=b_bn1_1, g12=g_bn1_2, b12=b_bn1_2,
                g21=g_bn2_1, b21=b_bn2_1, g22=g_bn2_2, b22=b_bn2_2,
                g3=g_bn3, b3=b_bn3).items()}
    cores = list(range(NCORES))
    n_batch = float(T * B * HW)
    LAST_EXEC_NS.clear(); LAST_PASS_NAMES.clear(); LAST_PROFILES.clear()

    fr, fi = _dft()
    fwr_hi, fwr_lo = _hilo(_diag2(fr))
    fwi_hi, fwi_lo = _hilo(_diag2(fi))
    fh_ = np.stack([_diag2(fr), _diag2(fi), _diag2(-fi)])
    idb_ = np.eye(96).astype(_BF)
    idf_ = np.eye(96, dtype=np.float32)
    w1d = np.zeros((2, 128, 128), np.float32)
    for hf in range(2):
        for kk in range(4):
            w1d[hf, kk * 32:(kk + 1) * 32, kk * 32:(kk + 1) * 32] = w1[hf * 4 + kk]
    w1hi, w1lo = _hilo(w1d)

    # ---- pass 1 (batch-sharded)
    in1 = []
    for b in cores:
        in1.append({
            "x": np.ascontiguousarray(
                x[:, b].reshape(T, 2, 64, 2, H, W).transpose(0, 3, 5, 2, 1, 4)
                .reshape(T, 96, NF)),
            "fw_hi": np.stack([fwr_hi, fwi_hi]), "fw_lo": np.stack([fwr_lo, fwi_lo]),
            "fh": fh_, "idb": idb_, "idf": idf_, "w1hi": w1hi, "w1lo": w1lo,
        })
    r1 = _run("p1", _get_nc("p1"), in1, cores)

    # ---- BN1 stats on host (f64, identical to original 4-pass flow)
    sum_m = sum(r["amb_d"].sum(axis=(0, 2), dtype=np.float64) for r in r1)
    sum_p = sum(r["apb_d"].sum(axis=(0, 2), dtype=np.float64) for r in r1)
    ssq_m = sum(np.einsum('tcs,tcs->c', r["amb_d"], r["amb_d"],
                          dtype=np.float64) for r in r1)
    ssq_p = sum(np.einsum('tcs,tcs->c', r["apb_d"], r["apb_d"],
                          dtype=np.float64) for r in r1)
    sclm, shm = _bn_affine(sum_m, ssq_m, gb_["g11"], gb_["b11"], n_batch)
    sclp, shp = _bn_affine(sum_p, ssq_p, gb_["g12"], gb_["b12"], n_batch)

    # ---- reshard to channel blocks with (v,u) free order
    def reshard(arrs, dt):
        # arrs: list over b of [T, C, HW] -> per-core-k [T, 2, 128, HW] (v,u)
        full = np.stack(arrs, axis=1)          # [T, B, C, HW]
        full = full.reshape(T, B, C, H, W).transpose(0, 1, 2, 4, 3)  # (v,u)
        out = []
        for k in range(NCORES):
            blk = full[:, :, 32 * k:32 * k + 32]            # [T, B, 32, W, H]
            blk = blk.reshape(T, 2, 4, 32, HW).reshape(T, 2, 128, HW)
            out.append(np.ascontiguousarray(blk.astype(dt)))
        return out

    amb_k = reshard([r["amb_d"] for r in r1], np.float32)
    apb_k = reshard([r["apb_d"] for r in r1], np.float32)
    sd_k = reshard([r["so_d"] for r in r1], _BF)
    ss_k = reshard([r["se_d"] for r in r1], _BF)

    ssel_ = np.zeros((128, 32), np.float32)
    for p in range(128):
        ssel_[p, p % 32] = 1.0
    fwb_ = np.stack([_diag2(fr), _diag2(fi)]).astype(_BF)
    fhb_ = np.stack([_diag2(fr), _diag2(-fi)]).astype(_BF)

    inB = []
    for k in cores:
        ch = slice(32 * k, 32 * k + 32)
        bn1_ = np.stack([np.tile(sclm[ch], 4), np.tile(shm[ch], 4),
                         np.tile(sclp[ch], 4), np.tile(shp[ch], 4)])
        w1d4_ = np.zeros((128, 128), np.float32)
        for m in range(4):
            w1d4_[m * 32:(m + 1) * 32, m * 32:(m + 1) * 32] = w1[k]
        gbv = np.stack([gb_["g21"][ch], gb_["b21"][ch], gb_["g22"][ch],
                        gb_["b22"][ch], gb_["g3"][ch], gb_["b3"][ch]])
        inB.append({"amb_B": amb_k[k], "apb_B": apb_k[k], "sd_B": sd_k[k],
                    "ss_B": ss_k[k], "bn1": bn1_, "w1d4": w1d4_.astype(_BF),
                    "fwb": fwb_, "fhb": fhb_, "idb": idb_, "gb": gbv,
                    "ssel": ssel_})
    rB = _run("pB", _get_nc("pB"), inB, cores)

    out = np.empty((T, B, C, H, W), np.float32)
    for k in cores:
        ob = rB[k]["out_B"].reshape(T, 2, 4, 32, H, W)
        for g in range(2):
            for m in range(4):
                out[:, 4 * g + m, 32 * k:32 * k + 32] = ob[:, g, m]
    return out


# revision 17
# speedup vs baseline: 1.0299x; 1.0024x over previous
"""Trainium2 Bass kernel for the spiking spectral net (nn_ASFF).

Pipeline: LIF -> FFT2 -> LIF -> blockdiag matmul -> BN -> LIF -> blockdiag
matmul -> BN -> combine -> softshrink -> FFT2.real -> BN.

Sharding: data-parallel over B (8 samples -> 8 cores). Four SPMD NEFF passes
with host-side all-reduce of BatchNorm statistics between them (stats are
[C]-vectors; everything heavy stays on device).

Layout notes:
 - c-layout: [128 partitions = half of C, 2304 free = (u,v) flattened hw]
 - spatial layout: [96 partitions = (d,h) with d = C-half, 6144 free]
 - FFT2 per 48x48 tile is done as W-side DFT (contract w), TensorE
   transpose, H-side DFT (contract h). DFT matrices are symmetric.
 - Matmuls on spike inputs use bf16 hi/lo-split DFT/weight matrices
   (exact to ~2^-17); the second FFT side has continuous input and uses
   exact fp32 matmuls. Post-threshold math (pass 3) is bf16 throughout.
"""
import sys
sys.path.insert(0, '/opt/trn_rl_repo')
import numpy as np
import ml_dtypes
import concourse.bass as bass
import concourse.tile as tile
import concourse.mybir as mybir
from concourse.bass_utils import run_bass_kernel_spmd

f32, bf16, f32r = mybir.dt.float32, mybir.dt.bfloat16, mybir.dt.float32r
AOT = mybir.AluOpType
AFT = mybir.ActivationFunctionType

T, B, C, H, W = 4, 8, 256, 48, 48
K, BS = 8, 32
HW = H * W            # 2304
NCORES = 8
NP = 96               # spatial-layout partitions (2 c-halves x 48)
NF = 6144             # spatial-layout free size (64 pairs x 2 x 48)
LAM = 0.06
EPS = 1e-5
NCHUNK = [480] * 12 + [384]           # spatial free chunking (5/4 pairs each)
CCHUNK = [512, 512, 512, 512, 256]    # c-layout free chunking of 2304

_BF = ml_dtypes.bfloat16
SBIG = float(2 ** 30)


def _hilo(x):
    hi = x.astype(np.float32).astype(_BF)
    lo = (x.astype(np.float32) - hi.astype(np.float32)).astype(_BF)
    return hi, lo


def _dft():
    j = np.arange(48)
    ang = -2.0 * np.pi * np.outer(j, j) / 48.0
    fr = (np.cos(ang) / np.sqrt(48.0)).astype(np.float32)
    fi = (np.sin(ang) / np.sqrt(48.0)).astype(np.float32)
    return fr, fi


def _diag2(m):
    out = np.zeros((96, 96), m.dtype)
    out[:48, :48] = m
    out[48:, 48:] = m
    return out


def split_waits(nc, max_waits=1):
    """This toolchain's walrus only tolerates one sync-wait per instruction;
    spill extra waits onto NoOps inserted just before the instruction."""
    ctr = 0
    for f in nc.m.functions:
        for bb in f.blocks:
            insts = list(bb.instructions)
            out = []
            changed = False
            for inst in insts:
                si = inst.sync_info
                waits = list(si.on_wait) if si else []
                if len(waits) > max_waits:
                    for wcond in waits[:-max_waits]:
                        ctr += 1
                        nop = mybir.InstNoOp(name=f"wsplit-{ctr}")
                        nop.engine = inst.engine
                        nop.sync_info = mybir.SyncInfo(on_wait=[wcond], on_update=[])
                        out.append(nop)
                    si.on_wait = waits[-max_waits:]
                    changed = True
                out.append(inst)
            if changed:
                bb.instructions = out
    return ctr


def _lif_step(nc, w_state, x_ap, s_out, ns_scratch=None):
    """One LIF step on w = 2*v scaled state: u = 0.5*w + x (into w_state),
    s = (u >= 1), w = min(u,1) - s  (== u*(u<1) bitwise).  x_ap may be PSUM."""
    nc.vector.scalar_tensor_tensor(w_state, w_state, 0.5, x_ap, AOT.mult, AOT.add)
    nc.vector.tensor_scalar(s_out, w_state, 1.0, None, AOT.is_ge)
    nc.vector.scalar_tensor_tensor(w_state, w_state, 1.0, s_out, AOT.min, AOT.subtract)


# ---------------------------------------------------------------- pass 1
def build_pass1():
    nc = bass.Bass()
    x = nc.declare_dram_parameter("x", [T, 96, NF], f32, isOutput=False)
    fw_hi = nc.declare_dram_parameter("fw_hi", [2, 96, 96], bf16, isOutput=False)
    fw_lo = nc.declare_dram_parameter("fw_lo", [2, 96, 96], bf16, isOutput=False)
    fh = nc.declare_dram_parameter("fh", [3, 96, 96], f32, isOutput=False)  # Fr2, Fi2, -Fi2
    idb = nc.declare_dram_parameter("idb", [96, 96], bf16, isOutput=False)
    idf = nc.declare_dram_parameter("idf", [96, 96], f32, isOutput=False)
    w1hi = nc.declare_dram_parameter("w1hi", [2, 128, 128], bf16, isOutput=False)
    w1lo = nc.declare_dram_parameter("w1lo", [2, 128, 128], bf16, isOutput=False)
    so_d = nc.declare_dram_parameter("so_d", [T, C, HW], bf16, isOutput=True)  # sd = so - se
    se_d = nc.declare_dram_parameter("se_d", [T, C, HW], bf16, isOutput=True)  # ss = so + se
    amb_d = nc.declare_dram_parameter("amb_d", [T, C, HW], f32, isOutput=True)
    apb_d = nc.declare_dram_parameter("apb_d", [T, C, HW], f32, isOutput=True)

    with tile.TileContext(nc) as tc:
        with tc.tile_pool(name="const", bufs=1) as cpool, \
             tc.tile_pool(name="state", bufs=1) as spool, \
             tc.tile_pool(name="work", bufs=2) as wpool, \
             tc.tile_pool(name="xtp", bufs=1) as xtp, \
             tc.tile_pool(name="ps", bufs=2, space="PSUM") as ps, \
             tc.tile_pool(name="pst", bufs=2, space="PSUM") as pst:

            fwr_hi = cpool.tile([96, 96], bf16); nc.sync.dma_start(fwr_hi[:], fw_hi[0])
            fwi_hi = cpool.tile([96, 96], bf16); nc.sync.dma_start(fwi_hi[:], fw_hi[1])
            fwr_lo = cpool.tile([96, 96], bf16); nc.sync.dma_start(fwr_lo[:], fw_lo[0])
            fwi_lo = cpool.tile([96, 96], bf16); nc.sync.dma_start(fwi_lo[:], fw_lo[1])
            fhr = cpool.tile([96, 96], f32); nc.sync.dma_start(fhr[:], fh[0])
            fhi = cpool.tile([96, 96], f32); nc.sync.dma_start(fhi[:], fh[1])
            fhin = cpool.tile([96, 96], f32); nc.sync.dma_start(fhin[:], fh[2])
            idbt = cpool.tile([96, 96], bf16); nc.sync.dma_start(idbt[:], idb[:])
            idft = cpool.tile([96, 96], f32); nc.sync.dma_start(idft[:], idf[:])
            w1t = []
            for hf in range(2):
                whi = cpool.tile([128, 128], bf16, tag=f"whi{hf}")
                nc.sync.dma_start(whi[:], w1hi[hf])
                wlo = cpool.tile([128, 128], bf16, tag=f"wlo{hf}")
                nc.sync.dma_start(wlo[:], w1lo[hf])
                w1t.append((whi, wlo))

            w1s = spool.tile([NP, NF], f32); nc.vector.memset(w1s[:], 0.0)
            wr = spool.tile([NP, NF], f32); nc.vector.memset(wr[:], 0.0)
            wi = spool.tile([NP, NF], f32); nc.vector.memset(wi[:], 0.0)

            for t in range(T):
                # ---- load x[t] in spatial layout [(d,h), (c',w)]; one DMA per d
                xt = xtp.tile([NP, NF], f32, tag="xt")
                nc.sync.dma_start(xt[:], x[t])

                # ---- fused chunk loop: LIF1 -> T1 -> W1 -> T2 -> H2 -> LIF2 -> store
                off = 0
                for g in range(13):
                    cw = NCHUNK[g]
                    npairs = cw // 96
                    sl = slice(off, off + cw)
                    # LIF1 chunk
                    s_c = wpool.tile([96, 480], bf16, tag="s_c")
                    nsc0 = wpool.tile([96, 480], f32, tag="nsc0")
                    _lif_step(nc, w1s[:, sl], xt[:, sl], s_c[:, :cw], nsc0[:, :cw])
                    # W1 (contract w, bf16 hi/lo) directly on spikes
                    pr_ = ps.tile([96, 480], f32, tag="mm0")
                    nc.tensor.matmul(pr_[:, :cw], fwr_hi[:], s_c[:, :cw], start=True, stop=False)
                    nc.tensor.matmul(pr_[:, :cw], fwr_lo[:], s_c[:, :cw], start=False, stop=True)
                    pi_ = ps.tile([96, 480], f32, tag="mm1")
                    nc.tensor.matmul(pi_[:, :cw], fwi_hi[:], s_c[:, :cw], start=True, stop=False)
                    nc.tensor.matmul(pi_[:, :cw], fwi_lo[:], s_c[:, :cw], start=False, stop=True)
                    xw_r = wpool.tile([96, 480], f32, tag="xw_r")
                    xw_i = wpool.tile([96, 480], f32, tag="xw_i")
                    nc.scalar.copy(xw_r[:, :cw], pr_[:, :cw])
                    nc.vector.tensor_copy(xw_i[:, :cw], pi_[:, :cw])
                    # T2 back to [(d,h), ...] fp32, batched into one psum tile per tensor
                    pt2r = pst.tile([96, 480], f32, tag="tp")
                    for j in range(npairs):
                        js = slice(j * 96, (j + 1) * 96)
                        nc.tensor.transpose(pt2r[:, js], xw_r[:, js], idft[:])
                    xwtr = wpool.tile([96, 480], f32, tag="xwtr")
                    nc.scalar.copy(xwtr[:, :cw], pt2r[:, :cw])
                    pt2i = pst.tile([96, 480], f32, tag="tp")
                    for j in range(npairs):
                        js = slice(j * 96, (j + 1) * 96)
                        nc.tensor.transpose(pt2i[:, js], xw_i[:, js], idft[:])
                    xwti = wpool.tile([96, 480], f32, tag="xwti")
                    nc.vector.tensor_copy(xwti[:, :cw], pt2i[:, :cw])
                    # H2 (contract h, exact fp32) + LIF2 fused
                    pre = ps.tile([96, 480], f32, tag="mm0")
                    nc.tensor.matmul(pre[:, :cw], fhr[:], xwtr[:, :cw], start=True, stop=False)
                    nc.tensor.matmul(pre[:, :cw], fhin[:], xwti[:, :cw], start=False, stop=True)
                    pim = ps.tile([96, 480], f32, tag="mm1")
                    nc.tensor.matmul(pim[:, :cw], fhi[:], xwtr[:, :cw], start=True, stop=False)
                    nc.tensor.matmul(pim[:, :cw], fhr[:], xwti[:, :cw], start=False, stop=True)
                    so_c = wpool.tile([96, 480], bf16, tag="so_cc")
                    se_c = wpool.tile([96, 480], bf16, tag="se_cc")
                    _lif_step(nc, wr[:, sl], pre[:, :cw], so_c[:, :cw])
                    _lif_step(nc, wi[:, sl], pim[:, :cw], se_c[:, :cw])
                    sd_c = wpool.tile([96, 480], bf16, tag="sd_cc")
                    ss_c = wpool.tile([96, 480], bf16, tag="ss_cc")
                    nc.vector.tensor_tensor(sd_c[:, :cw], so_c[:, :cw], se_c[:, :cw], AOT.subtract)
                    nc.vector.tensor_tensor(ss_c[:, :cw], so_c[:, :cw], se_c[:, :cw], AOT.add)
                    # store spike-diff chunks to DRAM in [c][u][v] order
                    p0 = off // 96  # first pair index of chunk
                    for (tile_, dram) in ((sd_c, so_d), (ss_c, se_d)):
                        for d in range(2):
                            c0 = d * 128 + p0 * 2
                            dst2 = dram[t, c0:c0 + npairs * 2].rearrange(
                                "(pr ct) (u v) -> u pr ct v", ct=2, v=48)
                            src2 = tile_[d * 48:(d + 1) * 48, :cw].rearrange(
                                "u (pr ct v) -> u pr ct v", ct=2, v=48)
                            nc.sync.dma_start(dst2, src2)
                    off += cw

                # ---- einsum1 directly on sd/ss from c-layout reload
                for hf in range(2):
                    sd_r = wpool.tile([128, HW], bf16, tag="so_r", bufs=1)
                    nc.sync.dma_start(sd_r[:], so_d[t, hf * 128:(hf + 1) * 128, :])
                    ss_r = wpool.tile([128, HW], bf16, tag="se_r", bufs=1)
                    nc.sync.dma_start(ss_r[:], se_d[t, hf * 128:(hf + 1) * 128, :])
                    whi, wlo = w1t[hf]
                    off2 = 0
                    for ci, cw in enumerate(CCHUNK):
                        sl = slice(off2, off2 + cw)
                        pa = ps.tile([128, 512], f32, tag="mm0")
                        nc.tensor.matmul(pa[:, :cw], whi[:], sd_r[:, sl], start=True, stop=False)
                        nc.tensor.matmul(pa[:, :cw], wlo[:], sd_r[:, sl], start=False, stop=True)
                        pb = ps.tile([128, 512], f32, tag="mm1")
                        nc.tensor.matmul(pb[:, :cw], whi[:], ss_r[:, sl], start=True, stop=False)
                        nc.tensor.matmul(pb[:, :cw], wlo[:], ss_r[:, sl], start=False, stop=True)
                        amb = wpool.tile([128, 512], f32, tag="amb")
                        apb = wpool.tile([128, 512], f32, tag="apb")
                        nc.scalar.copy(amb[:, :cw], pa[:, :cw])
                        nc.vector.tensor_copy(apb[:, :cw], pb[:, :cw])
                        nc.sync.dma_start(amb_d[t, hf * 128:(hf + 1) * 128, sl], amb[:, :cw])
                        nc.sync.dma_start(apb_d[t, hf * 128:(hf + 1) * 128, sl], apb[:, :cw])
                        off2 += cw

    split_waits(nc)
    return nc


# ------------------------------------------------------- fused pass B (2+3+4)
# Channel-block sharded: core k owns channels [32k, 32k+32) for ALL samples.
# c-layout tiles: [128 = (4 samples x 32 ch), 2304 free].  Free order is (v,u)
# for everything before FFT3 so the spatial reload needs no pre-transpose.
NB = T * B * HW * 1.0   # per-channel count for BN stats


def build_passB():
    nc = bass.Bass()
    amb_B = nc.declare_dram_parameter("amb_B", [T, 2, 128, HW], f32, isOutput=False)
    apb_B = nc.declare_dram_parameter("apb_B", [T, 2, 128, HW], f32, isOutput=False)
    sd_B = nc.declare_dram_parameter("sd_B", [T, 2, 128, HW], bf16, isOutput=False)
    ss_B = nc.declare_dram_parameter("ss_B", [T, 2, 128, HW], bf16, isOutput=False)
    bn1 = nc.declare_dram_parameter("bn1", [4, 128], f32, isOutput=False)  # sclm,shm,sclp,shp
    w1d4 = nc.declare_dram_parameter("w1d4", [128, 128], bf16, isOutput=False)
    fwb = nc.declare_dram_parameter("fwb", [2, 96, 96], bf16, isOutput=False)
    fhb = nc.declare_dram_parameter("fhb", [2, 96, 96], bf16, isOutput=False)
    idb = nc.declare_dram_parameter("idb", [96, 96], bf16, isOutput=False)
    idf = nc.declare_dram_parameter("idf", [96, 96], f32, isOutput=False)
    gb = nc.declare_dram_parameter("gb", [6, 32], f32, isOutput=False)  # g21,b21,g22,b22,g3,b3
    ssel = nc.declare_dram_parameter("ssel", [128, 32], f32, isOutput=False)
    out_B = nc.declare_dram_parameter("out_B", [T, 2, 128, HW], f32, isOutput=True)
    y_d = nc.dram_tensor("y_d", [T, B, 32, 48, 48], bf16)       # (v,u) free order
    a2m_d = nc.dram_tensor("a2m_d", [T, 2, 128, HW], bf16)
    a2p_d = nc.dram_tensor("a2p_d", [T, 2, 128, HW], bf16)
    yf_d = nc.dram_tensor("yf_d", [T, B, 32, 48, 48], bf16)     # (u,v) free order
    invN = 1.0 / NB

    def bn_finalize(wpool, stats4, gcol, scl128s, sh128s, n_streams=2):
        """stats4: [32,4] sbuf cols (sum_m, ssq_m, sum_p, ssq_p) already folded.
        Writes per-partition [128,1] scl/sh tiles per stream."""
        mean4 = wpool.tile([32, 4], f32, tag="mean4")
        nc.vector.tensor_scalar(mean4[:], stats4[:], invN, None, AOT.mult)
        for st_ in range(n_streams):
            mcol = mean4[:, 2 * st_:2 * st_ + 1]
            ecol = mean4[:, 2 * st_ + 1:2 * st_ + 2]
            m2 = wpool.tile([32, 1], f32, tag=f"m2_{st_}")
            nc.vector.tensor_tensor(m2[:], mcol, mcol, AOT.mult)
            var = wpool.tile([32, 1], f32, tag=f"var_{st_}")
            nc.vector.tensor_tensor(var[:], ecol, m2[:], AOT.subtract)
            nc.vector.tensor_scalar(var[:], var[:], EPS, None, AOT.add)
            std = wpool.tile([32, 1], f32, tag=f"std_{st_}")
            nc.scalar.activation(std[:], var[:], AFT.Sqrt)
            rstd = wpool.tile([32, 1], f32, tag=f"rstd_{st_}")
            nc.vector.reciprocal(rstd[:], std[:])
            scl32 = wpool.tile([32, 1], f32, tag=f"scl32_{st_}")
            nc.vector.tensor_tensor(scl32[:], gcol[2 * st_], rstd[:], AOT.mult)
            tmp = wpool.tile([32, 1], f32, tag=f"shtmp_{st_}")
            nc.vector.tensor_tensor(tmp[:], mcol, scl32[:], AOT.mult)
            sh32 = wpool.tile([32, 1], f32, tag=f"sh32_{st_}")
            nc.vector.tensor_tensor(sh32[:], gcol[2 * st_ + 1], tmp[:], AOT.subtract)
            for m in range(4):
                nc.sync.dma_start(scl128s[st_][32 * m:32 * m + 32, :], scl32[:])
                nc.sync.dma_start(sh128s[st_][32 * m:32 * m + 32, :], sh32[:])

    with tile.TileContext(nc) as tc:
        with tc.tile_pool(name="const", bufs=1) as cpool, \
             tc.tile_pool(name="bnsb", bufs=1) as bpool:
            bn1t = cpool.tile([128, 4], f32)
            nc.sync.dma_start(bn1t[:], bn1[:].rearrange("k p -> p k"))
            nbig = cpool.tile([128, 1], f32); nc.vector.memset(nbig[:], -SBIG)
            nlam = cpool.tile([128, 1], f32); nc.vector.memset(nlam[:], -LAM)
            w1t = cpool.tile([128, 128], bf16); nc.sync.dma_start(w1t[:], w1d4[:])
            fwr = cpool.tile([96, 96], bf16); nc.sync.dma_start(fwr[:], fwb[0])
            fwi = cpool.tile([96, 96], bf16); nc.sync.dma_start(fwi[:], fwb[1])
            fhr = cpool.tile([96, 96], bf16); nc.sync.dma_start(fhr[:], fhb[0])
            fhin = cpool.tile([96, 96], bf16); nc.sync.dma_start(fhin[:], fhb[1])
            idbt = cpool.tile([96, 96], bf16); nc.sync.dma_start(idbt[:], idb[:])
            idft = cpool.tile([96, 96], f32); nc.sync.dma_start(idft[:], idf[:])
            gbt = cpool.tile([32, 6], f32)
            nc.sync.dma_start(gbt[:], gb[:].rearrange("k c -> c k"))
            sselt = cpool.tile([128, 32], f32); nc.sync.dma_start(sselt[:], ssel[:])
            # stat collectors (col = t*2+g) and bn tiles
            coll = {}
            for nm in ("sum_m", "ssq_m", "sum_p", "ssq_p", "sum_3", "ssq_3"):
                ncol = 40 if nm in ("sum_m", "sum_p") else 8
                coltile = bpool.tile([128, ncol], f32, tag=f"coll_{nm}")
                coll[nm] = coltile
            scl2 = []
            sh2 = []
            for s in range(2):
                sctile = bpool.tile([128, 1], f32, tag=f"scl2_{s}")
                scl2.append(sctile)
                shtile = bpool.tile([128, 1], f32, tag=f"sh2_{s}")
                sh2.append(shtile)
            scl3t = bpool.tile([128, 1], f32, tag="scl3")
            sh3t = bpool.tile([128, 1], f32, tag="sh3")
            scl3 = [scl3t]
            sh3 = [sh3t]

            with tc.tile_pool(name="a2res", bufs=1) as a2pool:

                # ---------------- stage 2: BN1 affine -> LIF3 -> einsum2 + stats
                with tc.tile_pool(name="s2w", bufs=2) as wpool, \
                     tc.tile_pool(name="s2st", bufs=1) as spool, \
                     tc.tile_pool(name="s2ps", bufs=2, space="PSUM") as ps:
                    w3 = {}
                    for g in range(2):
                        for st_ in range(2):
                            wtile = spool.tile([128, HW], f32, tag=f"w3_{g}_{st_}")
                            w3[g, st_] = wtile
                            nc.vector.memset(wtile[:], 0.0)
                    for t in range(T):
                        for g in range(2):
                            col = t * 2 + g
                            o1 = []
                            for st_, dram in ((0, amb_B), (1, apb_B)):
                                at = wpool.tile([128, HW], f32, tag=f"at{st_}")
                                nc.sync.dma_start(at[:], dram[t, g])
                                kb = st_ * 2
                                pre = wpool.tile([128, HW], f32, tag=f"pre{st_}")
                                nc.vector.tensor_scalar(pre[:], at[:],
                                                        bn1t[:, kb:kb + 1],
                                                        bn1t[:, kb + 1:kb + 2],
                                                        AOT.mult, AOT.add)
                                w = w3[g, st_]
                                nc.vector.scalar_tensor_tensor(w[:], w[:], 0.5, pre[:],
                                                               AOT.mult, AOT.add)
                                sbf = wpool.tile([128, HW], bf16, tag=f"sbf{st_}")
                                nc.scalar.activation(sbf[:], w[:], AFT.Sigmoid,
                                                     bias=nbig[:], scale=SBIG)
                                nc.vector.scalar_tensor_tensor(w[:], w[:], 1.0, w[:],
                                                               AOT.is_lt, AOT.mult)
                                o1.append(sbf)
                            o1d = wpool.tile([128, HW], bf16, tag="o1d")
                            nc.vector.tensor_tensor(o1d[:], o1[0][:], o1[1][:], AOT.subtract)
                            o1a = wpool.tile([128, HW], bf16, tag="o1a")
                            nc.vector.tensor_tensor(o1a[:], o1[0][:], o1[1][:], AOT.add)
                            a2m = wpool.tile([128, HW], bf16, tag="a2m")
                            a2p = wpool.tile([128, HW], bf16, tag="a2p")
                            off2 = 0
                            for ci, cw in enumerate(CCHUNK):
                                sl = slice(off2, off2 + cw)
                                ccol = col * 5 + ci
                                pa = ps.tile([128, 512], f32, tag="mm0")
                                nc.tensor.matmul(pa[:, :cw], w1t[:], o1d[:, sl], start=True, stop=True)
                                pb = ps.tile([128, 512], f32, tag="mm1")
                                nc.tensor.matmul(pb[:, :cw], w1t[:], o1a[:, sl], start=True, stop=True)
                                nc.vector.tensor_scalar(
                                    a2m[:, sl], pa[:, :cw], 0.0, 0.0, AOT.add,
                                    AOT.add, accum_out=coll["sum_m"][:, ccol:ccol + 1])
                                nc.scalar.activation(
                                    a2p[:, sl], pb[:, :cw], AFT.Copy,
                                    accum_out=coll["sum_p"][:, ccol:ccol + 1])
                                off2 += cw
                            # ssq stats for BN2 on ScalarE
                            sqs = wpool.tile([128, HW], bf16, tag="sqs")
                            nc.scalar.activation(sqs[:], a2m[:], AFT.Square,
                                                 accum_out=coll["ssq_m"][:, col:col + 1])
                            sqs2 = wpool.tile([128, HW], bf16, tag="sqs")
                            nc.scalar.activation(sqs2[:], a2p[:], AFT.Square,
                                                 accum_out=coll["ssq_p"][:, col:col + 1])
                            nc.sync.dma_start(a2m_d[t, g], a2m[:])
                            nc.sync.dma_start(a2p_d[t, g], a2p[:])

                # ---------------- BN2 stats finalize
                with tc.tile_pool(name="bnf", bufs=1) as fpool, \
                     tc.tile_pool(name="bnfps", bufs=1, space="PSUM") as fps:
                    stats4p = fpool.tile([128, 4], f32, tag="stats4p")
                    for i, nm in enumerate(("sum_m", "ssq_m", "sum_p", "ssq_p")):
                        nc.vector.tensor_reduce(stats4p[:, i:i + 1], coll[nm][:],
                                                mybir.AxisListType.X, AOT.add)
                    pf = fps.tile([32, 4], f32, tag="fold")
                    nc.tensor.matmul(pf[:], sselt[:], stats4p[:], start=True, stop=True)
                    stats4 = fpool.tile([32, 4], f32, tag="stats4")
                    nc.vector.tensor_copy(stats4[:], pf[:])
                    gcol = [gbt[:, i:i + 1] for i in range(4)]
                    bn_finalize(fpool, stats4, gcol, scl2, sh2)

                # ---------------- stage 3: BN2 affine + combine + softshrink + FFT3
                with tc.tile_pool(name="s3w", bufs=2) as wpool, \
                     tc.tile_pool(name="s3f", bufs=2) as fw_, \
                     tc.tile_pool(name="psW", bufs=2, space="PSUM") as psW, \
                     tc.tile_pool(name="psH", bufs=2, space="PSUM") as psH, \
                     tc.tile_pool(name="pst", bufs=2, space="PSUM") as pst:
                    for t in range(T):
                        for g in range(2):
                            a2mt = wpool.tile([128, HW], bf16, tag="a2mt")
                            nc.sync.dma_start(a2mt[:], a2m_d[t, g])
                            a2pt = wpool.tile([128, HW], bf16, tag="a2pt")
                            nc.sync.dma_start(a2pt[:], a2p_d[t, g])
                            o2m = wpool.tile([128, HW], bf16, tag="o2m")
                            nc.scalar.activation(o2m[:], a2mt[:], AFT.Identity,
                                                 bias=sh2[0][:], scale=scl2[0][:])
                            o2p = wpool.tile([128, HW], bf16, tag="o2p")
                            nc.scalar.activation(o2p[:], a2pt[:], AFT.Identity,
                                                 bias=sh2[1][:], scale=scl2[1][:])
                            sdt = wpool.tile([128, HW], bf16, tag="sdt")
                            nc.sync.dma_start(sdt[:], sd_B[t, g])
                            sst = wpool.tile([128, HW], bf16, tag="sst")
                            nc.sync.dma_start(sst[:], ss_B[t, g])
                            m1 = wpool.tile([128, HW], bf16, tag="m1")
                            nc.vector.tensor_tensor(m1[:], sdt[:], o2m[:], AOT.mult)
                            m2_ = wpool.tile([128, HW], bf16, tag="m2c")
                            nc.vector.tensor_tensor(m2_[:], sst[:], o2p[:], AOT.mult)
                            yc = wpool.tile([128, HW], bf16, tag="yc")
                            nc.vector.tensor_tensor(yc[:], m1[:], m2_[:], AOT.subtract)
                            r1 = wpool.tile([128, HW], bf16, tag="r1")
                            nc.scalar.activation(r1[:], yc[:], AFT.Relu, bias=nlam[:], scale=1.0)
                            r2 = wpool.tile([128, HW], bf16, tag="r2")
                            nc.scalar.activation(r2[:], yc[:], AFT.Relu, bias=nlam[:], scale=-1.0)
                            nc.vector.tensor_tensor(yc[:], r1[:], r2[:], AOT.subtract)
                            dst = y_d[t, 4 * g:4 * g + 4].rearrange("m c v u -> (m c) (v u)")
                            nc.sync.dma_start(dst, yc[:])

                        # FFT3 per sample pair (contract v, transpose, contract u)
                        for sp in range(4):
                            yt = fw_.tile([96, 1536], bf16, tag="yt")
                            for m in range(2):
                                nc.sync.dma_start(
                                    yt[48 * m:48 * m + 48, :].rearrange(
                                        "v (c u) -> v c u", c=32),
                                    y_d[t, 2 * sp + m].rearrange("c v u -> v c u"))
                            yfs = fw_.tile([96, 1536], bf16, tag="yfs")
                            off = 0
                            for ci, cw in enumerate((480, 480, 480, 96)):
                                npairs = cw // 96
                                sl = slice(off, off + cw)
                                pw_re = psW.tile([96, 480], f32, tag="mm0")
                                nc.tensor.matmul(pw_re[:, :cw], fwr[:], yt[:, sl], start=True, stop=True)
                                pw_im = psW.tile([96, 480], f32, tag="mm1")
                                nc.tensor.matmul(pw_im[:, :cw], fwi[:], yt[:, sl], start=True, stop=True)
                                # pack re (even bf16 slots) and im (odd) into one f32 tile
                                ywp = wpool.tile([96, 480], f32, tag="ywp")
                                ywp_v = ywp[:].bitcast(bf16).rearrange(
                                    "p (f two) -> p f two", two=2)
                                nc.scalar.copy(ywp_v[:, :cw, 0], pw_re[:, :cw])
                                nc.vector.tensor_copy(ywp_v[:, :cw, 1], pw_im[:, :cw])
                                pt2 = pst.tile([96, 480], f32, tag="tp")
                                for j in range(npairs):
                                    js = slice(j * 96, (j + 1) * 96)
                                    nc.tensor.transpose(pt2[:, js], ywp[:, js], idft[:])
                                pt2_v = pt2[:].bitcast(bf16).rearrange(
                                    "p (f two) -> p f two", two=2)
                                ywtr = wpool.tile([96, 480], bf16, tag="ywtr")
                                nc.vector.tensor_copy(ywtr[:, :cw], pt2_v[:, :cw, 0])
                                ywti = wpool.tile([96, 480], bf16, tag="ywti")
                                nc.scalar.copy(ywti[:, :cw], pt2_v[:, :cw, 1])
                                ph = psH.tile([96, 480], f32, tag="mmh")
                                nc.tensor.matmul(ph[:, :cw], fhr[:], ywtr[:, :cw], start=True, stop=False)
                                nc.tensor.matmul(ph[:, :cw], fhin[:], ywti[:, :cw], start=False, stop=True)
                                if ci % 2 == 0:
                                    nc.scalar.copy(yfs[:, sl], ph[:, :cw])
                                else:
                                    nc.vector.tensor_copy(yfs[:, sl], ph[:, :cw])
                                off += cw
                            srcf = yfs[:].rearrange("p (cp m v) -> p cp m v", cp=16, m=2)
                            for m in range(2):
                                dstf = yf_d[t, 2 * sp + m].rearrange(
                                    "(cp c2) u v -> (c2 u) cp v", c2=2)
                                nc.sync.dma_start(dstf, srcf[:, :, m, :])

            # ---------------- stage 4: BN3 stats + affine + out
            with tc.tile_pool(name="yfres", bufs=1) as yfpool, \
                 tc.tile_pool(name="s4w", bufs=2) as wpool4, \
                 tc.tile_pool(name="s4ps", bufs=1, space="PSUM") as fps4:
                yf_res = {}
                for t in range(T):
                    for g in range(2):
                        col = t * 2 + g
                        yft = yfpool.tile([128, HW], bf16, tag=f"yf_{t}_{g}")
                        yf_res[t, g] = yft
                        nc.sync.dma_start(
                            yft[:], yf_d[t, 4 * g:4 * g + 4].rearrange("m c u v -> (m c) (u v)"))
                        sq3 = wpool4.tile([128, HW], bf16, tag="sq3")
                        nc.scalar.activation(sq3[:], yft[:], AFT.Square,
                                             accum_out=coll["ssq_3"][:, col:col + 1])
                        nc.vector.tensor_reduce(coll["sum_3"][:, col:col + 1], yft[:],
                                                mybir.AxisListType.X, AOT.add)
                stats4p3 = wpool4.tile([128, 4], f32, tag="stats4p3")
                nc.vector.memset(stats4p3[:], 0.0)
                for i, nm in enumerate(("sum_3", "ssq_3")):
                    nc.vector.tensor_reduce(stats4p3[:, i:i + 1], coll[nm][:],
                                            mybir.AxisListType.X, AOT.add)
                pf3 = fps4.tile([32, 4], f32, tag="fold3")
                nc.tensor.matmul(pf3[:], sselt[:], stats4p3[:], start=True, stop=True)
                stats43 = wpool4.tile([32, 4], f32, tag="stats43")
                nc.vector.tensor_copy(stats43[:], pf3[:])
                gcol3 = [gbt[:, 4:5], gbt[:, 5:6]]
                bn_finalize(wpool4, stats43, gcol3, scl3, sh3, n_streams=1)
                for t in range(T):
                    for g in range(2):
                        ot = wpool4.tile([128, HW], f32, tag="ot")
                        nc.scalar.activation(ot[:], yf_res[t, g][:], AFT.Identity,
                                             bias=sh3[0][:], scale=scl3[0][:])
                        nc.sync.dma_start(out_B[t, g], ot[:])
    split_waits(nc)
    return nc


# ---------------------------------------------------------------- host glue
_NCS = {}
LAST_EXEC_NS = []
LAST_PASS_NAMES = []
LAST_PROFILES = []


def _run(name, nc, in_maps, cores):
    r = run_bass_kernel_spmd(nc, in_maps, core_ids=cores)
    LAST_PASS_NAMES.append(name)
    LAST_EXEC_NS.append(r.exec_time_ns)
    LAST_PROFILES.append(r.profile_json)
    return r.results


def _get_nc(name):
    if name not in _NCS:
        _NCS[name] = {"p1": build_pass1, "pB": build_passB}[name]()
    return _NCS[name]


def _bn_affine(sums, ssqs, gamma, beta, n):
    mu = sums / n
    var = ssqs / n - mu * mu
    scl = gamma / np.sqrt(var + EPS)
    sh = beta - mu * scl
    return scl.astype(np.float32), sh.astype(np.float32)


def kernel(x, w1, g_bn1_1, b_bn1_1, g_bn1_2, b_bn1_2, g_bn2_1, b_bn2_1,
           g_bn2_2, b_bn2_2, g_bn3, b_bn3, alpha=None):
    x = np.asarray(x, np.float32)
    w1 = np.asarray(w1, np.float32)
    gb_ = {k: np.asarray(v, np.float32) for k, v in
           dict(g11=g_bn1_1, b11=b_bn1_1, g12=g_bn1_2, b12=b_bn1_2,
                g21=g_bn2_1, b21=b_bn2_1, g22=g_bn2_2, b22=b_bn2_2,
                g3=g_bn3, b3=b_bn3).items()}
    cores = list(range(NCORES))
    n_batch = float(T * B * HW)
    LAST_EXEC_NS.clear(); LAST_PASS_NAMES.clear(); LAST_PROFILES.clear()

    fr, fi = _dft()
    fwr_hi, fwr_lo = _hilo(_diag2(fr))
    fwi_hi, fwi_lo = _hilo(_diag2(fi))
    fh_ = np.stack([_diag2(fr), _diag2(fi), _diag2(-fi)])
    idb_ = np.eye(96).astype(_BF)
    idf_ = np.eye(96, dtype=np.float32)
    w1d = np.zeros((2, 128, 128), np.float32)
    for hf in range(2):
        for kk in range(4):
            w1d[hf, kk * 32:(kk + 1) * 32, kk * 32:(kk + 1) * 32] = w1[hf * 4 + kk]
    w1hi, w1lo = _hilo(w1d)

    # ---- pass 1 (batch-sharded)
    in1 = []
    for b in cores:
        in1.append({
            "x": np.ascontiguousarray(
                x[:, b].reshape(T, 2, 64, 2, H, W).transpose(0, 3, 5, 2, 1, 4)
                .reshape(T, 96, NF)),
            "fw_hi": np.stack([fwr_hi, fwi_hi]), "fw_lo": np.stack([fwr_lo, fwi_lo]),
            "fh": fh_, "idb": idb_, "idf": idf_, "w1hi": w1hi, "w1lo": w1lo,
        })
    r1 = _run("p1", _get_nc("p1"), in1, cores)

    # ---- BN1 stats on host (f64, identical to original 4-pass flow)
    sum_m = sum(r["amb_d"].sum(axis=(0, 2), dtype=np.float64) for r in r1)
    sum_p = sum(r["apb_d"].sum(axis=(0, 2), dtype=np.float64) for r in r1)
    ssq_m = sum(np.einsum('tcs,tcs->c', r["amb_d"], r["amb_d"],
                          dtype=np.float64) for r in r1)
    ssq_p = sum(np.einsum('tcs,tcs->c', r["apb_d"], r["apb_d"],
                          dtype=np.float64) for r in r1)
    sclm, shm = _bn_affine(sum_m, ssq_m, gb_["g11"], gb_["b11"], n_batch)
    sclp, shp = _bn_affine(sum_p, ssq_p, gb_["g12"], gb_["b12"], n_batch)

    # ---- reshard to channel blocks with (v,u) free order
    def reshard(arrs, dt):
        # arrs: list over b of [T, C, HW] -> per-core-k [T, 2, 128, HW] (v,u)
        full = np.stack(arrs, axis=1)          # [T, B, C, HW]
        full = full.reshape(T, B, C, H, W).transpose(0, 1, 2, 4, 3)  # (v,u)
        out = []
        for k in range(NCORES):
            blk = full[:, :, 32 * k:32 * k + 32]            # [T, B, 32, W, H]
            blk = blk.reshape(T, 2, 4, 32, HW).reshape(T, 2, 128, HW)
            out.append(np.ascontiguousarray(blk.astype(dt)))
        return out

    amb_k = reshard([r["amb_d"] for r in r1], np.float32)
    apb_k = reshard([r["apb_d"] for r in r1], np.float32)
    sd_k = reshard([r["so_d"] for r in r1], _BF)
    ss_k = reshard([r["se_d"] for r in r1], _BF)

    ssel_ = np.zeros((128, 32), np.float32)
    for p in range(128):
        ssel_[p, p % 32] = 1.0
    fwb_ = np.stack([_diag2(fr), _diag2(fi)]).astype(_BF)
    fhb_ = np.stack([_diag2(fr), _diag2(-fi)]).astype(_BF)

    inB = []
    for k in cores:
        ch = slice(32 * k, 32 * k + 32)
        bn1_ = np.stack([np.tile(sclm[ch], 4), np.tile(shm[ch], 4),
                         np.tile(sclp[ch], 4), np.tile(shp[ch], 4)])
        w1d4_ = np.zeros((128, 128), np.float32)
        for m in range(4):
            w1d4_[m * 32:(m + 1) * 32, m * 32:(m + 1) * 32] = w1[k]
        gbv = np.stack([gb_["g21"][ch], gb_["b21"][ch], gb_["g22"][ch],
                        gb_["b22"][ch], gb_["g3"][ch], gb_["b3"][ch]])
        inB.append({"amb_B": amb_k[k], "apb_B": apb_k[k], "sd_B": sd_k[k],
                    "ss_B": ss_k[k], "bn1": bn1_, "w1d4": w1d4_.astype(_BF),
                    "fwb": fwb_, "fhb": fhb_, "idb": idb_, "idf": idf_, "gb": gbv,
                    "ssel": ssel_})
    rB = _run("pB", _get_nc("pB"), inB, cores)

    out = np.empty((T, B, C, H, W), np.float32)
    for k in cores:
        ob = rB[k]["out_B"].reshape(T, 2, 4, 32, H, W)
        for g in range(2):
            for m in range(4):
                out[:, 4 * g + m, 32 * k:32 * k + 32] = ob[:, g, m]
    return out


# revision 22
# speedup vs baseline: 1.0716x; 1.0405x over previous
"""Trainium2 Bass kernel for the spiking spectral net (nn_ASFF).

Pipeline: LIF -> FFT2 -> LIF -> blockdiag matmul -> BN -> LIF -> blockdiag
matmul -> BN -> combine -> softshrink -> FFT2.real -> BN.

Sharding: data-parallel over B (8 samples -> 8 cores). Four SPMD NEFF passes
with host-side all-reduce of BatchNorm statistics between them (stats are
[C]-vectors; everything heavy stays on device).

Layout notes:
 - c-layout: [128 partitions = half of C, 2304 free = (u,v) flattened hw]
 - spatial layout: [96 partitions = (d,h) with d = C-half, 6144 free]
 - FFT2 per 48x48 tile is done as W-side DFT (contract w), TensorE
   transpose, H-side DFT (contract h). DFT matrices are symmetric.
 - Matmuls on spike inputs use bf16 hi/lo-split DFT/weight matrices
   (exact to ~2^-17); the second FFT side has continuous input and uses
   exact fp32 matmuls. Post-threshold math (pass 3) is bf16 throughout.
"""
import sys
sys.path.insert(0, '/opt/trn_rl_repo')
import numpy as np
import ml_dtypes
import concourse.bass as bass
import concourse.tile as tile
import concourse.mybir as mybir
from concourse.bass_utils import run_bass_kernel_spmd

f32, bf16, f32r = mybir.dt.float32, mybir.dt.bfloat16, mybir.dt.float32r
AOT = mybir.AluOpType
AFT = mybir.ActivationFunctionType

T, B, C, H, W = 4, 8, 256, 48, 48
K, BS = 8, 32
HW = H * W            # 2304
NCORES = 8
NP = 96               # spatial-layout partitions (2 c-halves x 48)
NF = 6144             # spatial-layout free size (64 pairs x 2 x 48)
LAM = 0.06
EPS = 1e-5
NCHUNK = [480] * 12 + [384]           # spatial free chunking (5/4 pairs each)
CCHUNK = [512, 512, 512, 512, 256]    # c-layout free chunking of 2304

_BF = ml_dtypes.bfloat16
SBIG = float(2 ** 30)


def _hilo(x):
    hi = x.astype(np.float32).astype(_BF)
    lo = (x.astype(np.float32) - hi.astype(np.float32)).astype(_BF)
    return hi, lo


def _dft():
    j = np.arange(48)
    ang = -2.0 * np.pi * np.outer(j, j) / 48.0
    fr = (np.cos(ang) / np.sqrt(48.0)).astype(np.float32)
    fi = (np.sin(ang) / np.sqrt(48.0)).astype(np.float32)
    return fr, fi


def _diag2(m):
    out = np.zeros((96, 96), m.dtype)
    out[:48, :48] = m
    out[48:, 48:] = m
    return out


def split_waits(nc, max_waits=1):
    """This toolchain's walrus only tolerates one sync-wait per instruction;
    spill extra waits onto NoOps inserted just before the instruction."""
    ctr = 0
    for f in nc.m.functions:
        for bb in f.blocks:
            insts = list(bb.instructions)
            out = []
            changed = False
            for inst in insts:
                si = inst.sync_info
                waits = list(si.on_wait) if si else []
                if len(waits) > max_waits:
                    for wcond in waits[:-max_waits]:
                        ctr += 1
                        nop = mybir.InstNoOp(name=f"wsplit-{ctr}")
                        nop.engine = inst.engine
                        nop.sync_info = mybir.SyncInfo(on_wait=[wcond], on_update=[])
                        out.append(nop)
                    si.on_wait = waits[-max_waits:]
                    changed = True
                out.append(inst)
            if changed:
                bb.instructions = out
    return ctr


def _lif_step(nc, w_state, x_ap, s_out, ns_scratch=None):
    """One LIF step on w = 2*v scaled state: u = 0.5*w + x (into w_state),
    s = (u >= 1), w = min(u,1) - s  (== u*(u<1) bitwise).  x_ap may be PSUM."""
    nc.vector.scalar_tensor_tensor(w_state, w_state, 0.5, x_ap, AOT.mult, AOT.add)
    nc.vector.tensor_scalar(s_out, w_state, 1.0, None, AOT.is_ge)
    nc.vector.scalar_tensor_tensor(w_state, w_state, 1.0, s_out, AOT.min, AOT.subtract)


# ---------------------------------------------------------------- pass 1
def build_pass1():
    nc = bass.Bass()
    x = nc.declare_dram_parameter("x", [T, 96, NF], f32, isOutput=False)
    fw_hi = nc.declare_dram_parameter("fw_hi", [2, 96, 96], bf16, isOutput=False)
    fw_lo = nc.declare_dram_parameter("fw_lo", [2, 96, 96], bf16, isOutput=False)
    fh = nc.declare_dram_parameter("fh", [3, 96, 96], f32, isOutput=False)  # Fr2, Fi2, -Fi2
    idb = nc.declare_dram_parameter("idb", [96, 96], bf16, isOutput=False)
    idf = nc.declare_dram_parameter("idf", [96, 96], f32, isOutput=False)
    w1hi = nc.declare_dram_parameter("w1hi", [2, 128, 128], bf16, isOutput=False)
    w1lo = nc.declare_dram_parameter("w1lo", [2, 128, 128], bf16, isOutput=False)
    so_d = nc.declare_dram_parameter("so_d", [T, C, HW], bf16, isOutput=True)  # sd = so - se
    se_d = nc.declare_dram_parameter("se_d", [T, C, HW], bf16, isOutput=True)  # ss = so + se
    amb_d = nc.declare_dram_parameter("amb_d", [T, C, HW], f32, isOutput=True)
    apb_d = nc.declare_dram_parameter("apb_d", [T, C, HW], f32, isOutput=True)

    with tile.TileContext(nc) as tc:
        with tc.tile_pool(name="const", bufs=1) as cpool, \
             tc.tile_pool(name="state", bufs=1) as spool, \
             tc.tile_pool(name="work", bufs=2) as wpool, \
             tc.tile_pool(name="xtp", bufs=1) as xtp, \
             tc.tile_pool(name="ps", bufs=2, space="PSUM") as ps, \
             tc.tile_pool(name="pst", bufs=2, space="PSUM") as pst:

            fwr_hi = cpool.tile([96, 96], bf16); nc.sync.dma_start(fwr_hi[:], fw_hi[0])
            fwi_hi = cpool.tile([96, 96], bf16); nc.sync.dma_start(fwi_hi[:], fw_hi[1])
            fwr_lo = cpool.tile([96, 96], bf16); nc.sync.dma_start(fwr_lo[:], fw_lo[0])
            fwi_lo = cpool.tile([96, 96], bf16); nc.sync.dma_start(fwi_lo[:], fw_lo[1])
            fhr = cpool.tile([96, 96], f32); nc.sync.dma_start(fhr[:], fh[0])
            fhi = cpool.tile([96, 96], f32); nc.sync.dma_start(fhi[:], fh[1])
            fhin = cpool.tile([96, 96], f32); nc.sync.dma_start(fhin[:], fh[2])
            idbt = cpool.tile([96, 96], bf16); nc.sync.dma_start(idbt[:], idb[:])
            idft = cpool.tile([96, 96], f32); nc.sync.dma_start(idft[:], idf[:])
            w1t = []
            for hf in range(2):
                whi = cpool.tile([128, 128], bf16, tag=f"whi{hf}")
                nc.sync.dma_start(whi[:], w1hi[hf])
                wlo = cpool.tile([128, 128], bf16, tag=f"wlo{hf}")
                nc.sync.dma_start(wlo[:], w1lo[hf])
                w1t.append((whi, wlo))

            w1s = spool.tile([NP, NF], f32); nc.vector.memset(w1s[:], 0.0)
            wr = spool.tile([NP, NF], f32); nc.vector.memset(wr[:], 0.0)
            wi = spool.tile([NP, NF], f32); nc.vector.memset(wi[:], 0.0)

            for t in range(T):
                # ---- load x[t] in spatial layout [(d,h), (c',w)]; one DMA per d
                xt = xtp.tile([NP, NF], f32, tag="xt")
                nc.sync.dma_start(xt[:], x[t])

                # ---- fused chunk loop: LIF1 -> T1 -> W1 -> T2 -> H2 -> LIF2 -> store
                off = 0
                for g in range(13):
                    cw = NCHUNK[g]
                    npairs = cw // 96
                    sl = slice(off, off + cw)
                    # LIF1 chunk
                    s_c = wpool.tile([96, 480], bf16, tag="s_c")
                    _lif_step(nc, w1s[:, sl], xt[:, sl], s_c[:, :cw])
                    # W1 (contract w, bf16 hi/lo) directly on spikes
                    pr_ = ps.tile([96, 480], f32, tag="mm0")
                    nc.tensor.matmul(pr_[:, :cw], fwr_hi[:], s_c[:, :cw], start=True, stop=False)
                    nc.tensor.matmul(pr_[:, :cw], fwr_lo[:], s_c[:, :cw], start=False, stop=True)
                    pi_ = ps.tile([96, 480], f32, tag="mm1")
                    nc.tensor.matmul(pi_[:, :cw], fwi_hi[:], s_c[:, :cw], start=True, stop=False)
                    nc.tensor.matmul(pi_[:, :cw], fwi_lo[:], s_c[:, :cw], start=False, stop=True)
                    xw_r = wpool.tile([96, 480], f32, tag="xw_r")
                    xw_i = wpool.tile([96, 480], f32, tag="xw_i")
                    nc.scalar.copy(xw_r[:, :cw], pr_[:, :cw])
                    nc.vector.tensor_copy(xw_i[:, :cw], pi_[:, :cw])
                    # T2 back to [(d,h), ...] fp32, batched into one psum tile per tensor
                    pt2r = pst.tile([96, 480], f32, tag="tp")
                    for j in range(npairs):
                        js = slice(j * 96, (j + 1) * 96)
                        nc.tensor.transpose(pt2r[:, js], xw_r[:, js], idft[:])
                    xwtr = wpool.tile([96, 480], f32, tag="xwtr")
                    nc.scalar.copy(xwtr[:, :cw], pt2r[:, :cw])
                    pt2i = pst.tile([96, 480], f32, tag="tp")
                    for j in range(npairs):
                        js = slice(j * 96, (j + 1) * 96)
                        nc.tensor.transpose(pt2i[:, js], xw_i[:, js], idft[:])
                    xwti = wpool.tile([96, 480], f32, tag="xwti")
                    nc.vector.tensor_copy(xwti[:, :cw], pt2i[:, :cw])
                    # H2 (contract h, exact fp32) + LIF2 fused
                    pre = ps.tile([96, 480], f32, tag="mm0")
                    nc.tensor.matmul(pre[:, :cw], fhr[:], xwtr[:, :cw], start=True, stop=False)
                    nc.tensor.matmul(pre[:, :cw], fhin[:], xwti[:, :cw], start=False, stop=True)
                    pim = ps.tile([96, 480], f32, tag="mm1")
                    nc.tensor.matmul(pim[:, :cw], fhi[:], xwtr[:, :cw], start=True, stop=False)
                    nc.tensor.matmul(pim[:, :cw], fhr[:], xwti[:, :cw], start=False, stop=True)
                    so_c = wpool.tile([96, 480], bf16, tag="so_cc")
                    se_c = wpool.tile([96, 480], bf16, tag="se_cc")
                    _lif_step(nc, wr[:, sl], pre[:, :cw], so_c[:, :cw])
                    _lif_step(nc, wi[:, sl], pim[:, :cw], se_c[:, :cw])
                    sd_c = wpool.tile([96, 480], bf16, tag="sd_cc")
                    ss_c = wpool.tile([96, 480], bf16, tag="ss_cc")
                    nc.vector.tensor_tensor(sd_c[:, :cw], so_c[:, :cw], se_c[:, :cw], AOT.subtract)
                    nc.vector.tensor_tensor(ss_c[:, :cw], so_c[:, :cw], se_c[:, :cw], AOT.add)
                    # store spike-diff chunks to DRAM in [c][u][v] order
                    p0 = off // 96  # first pair index of chunk
                    for (tile_, dram) in ((sd_c, so_d), (ss_c, se_d)):
                        for d in range(2):
                            c0 = d * 128 + p0 * 2
                            dst2 = dram[t, c0:c0 + npairs * 2].rearrange(
                                "(pr ct) (u v) -> u pr ct v", ct=2, v=48)
                            src2 = tile_[d * 48:(d + 1) * 48, :cw].rearrange(
                                "u (pr ct v) -> u pr ct v", ct=2, v=48)
                            nc.sync.dma_start(dst2, src2)
                    off += cw

                # ---- einsum1 directly on sd/ss from c-layout reload
                for hf in range(2):
                    sd_r = wpool.tile([128, HW], bf16, tag="so_r", bufs=1)
                    nc.sync.dma_start(sd_r[:], so_d[t, hf * 128:(hf + 1) * 128, :])
                    ss_r = wpool.tile([128, HW], bf16, tag="se_r", bufs=1)
                    nc.sync.dma_start(ss_r[:], se_d[t, hf * 128:(hf + 1) * 128, :])
                    whi, wlo = w1t[hf]
                    off2 = 0
                    for ci, cw in enumerate(CCHUNK):
                        sl = slice(off2, off2 + cw)
                        pa = ps.tile([128, 512], f32, tag="mm0")
                        nc.tensor.matmul(pa[:, :cw], whi[:], sd_r[:, sl], start=True, stop=False)
                        nc.tensor.matmul(pa[:, :cw], wlo[:], sd_r[:, sl], start=False, stop=True)
                        pb = ps.tile([128, 512], f32, tag="mm1")
                        nc.tensor.matmul(pb[:, :cw], whi[:], ss_r[:, sl], start=True, stop=False)
                        nc.tensor.matmul(pb[:, :cw], wlo[:], ss_r[:, sl], start=False, stop=True)
                        amb = wpool.tile([128, 512], f32, tag="amb")
                        apb = wpool.tile([128, 512], f32, tag="apb")
                        nc.scalar.copy(amb[:, :cw], pa[:, :cw])
                        nc.vector.tensor_copy(apb[:, :cw], pb[:, :cw])
                        nc.sync.dma_start(amb_d[t, hf * 128:(hf + 1) * 128, sl], amb[:, :cw])
                        nc.sync.dma_start(apb_d[t, hf * 128:(hf + 1) * 128, sl], apb[:, :cw])
                        off2 += cw

    split_waits(nc)
    return nc


# ------------------------------------------------------- fused pass B (2+3+4)
# Channel-block sharded: core k owns channels [32k, 32k+32) for ALL samples.
# c-layout tiles: [128 = (4 samples x 32 ch), 2304 free].  Free order is (v,u)
# for everything before FFT3 so the spatial reload needs no pre-transpose.
NB = T * B * HW * 1.0   # per-channel count for BN stats


def build_passB():
    nc = bass.Bass()
    amb_B = nc.declare_dram_parameter("amb_B", [T, 2, 128, HW], f32, isOutput=False)
    apb_B = nc.declare_dram_parameter("apb_B", [T, 2, 128, HW], f32, isOutput=False)
    sd_B = nc.declare_dram_parameter("sd_B", [T, 2, 128, HW], bf16, isOutput=False)
    ss_B = nc.declare_dram_parameter("ss_B", [T, 2, 128, HW], bf16, isOutput=False)
    w1d4 = nc.declare_dram_parameter("w1d4", [128, 128], bf16, isOutput=False)
    w1d4n = nc.declare_dram_parameter("w1d4n", [128, 128], bf16, isOutput=False)
    fwb = nc.declare_dram_parameter("fwb", [2, 96, 96], bf16, isOutput=False)
    fhb = nc.declare_dram_parameter("fhb", [2, 96, 96], bf16, isOutput=False)
    idb = nc.declare_dram_parameter("idb", [96, 96], bf16, isOutput=False)
    idf = nc.declare_dram_parameter("idf", [96, 96], f32, isOutput=False)
    gb = nc.declare_dram_parameter("gb", [6, 32], f32, isOutput=False)  # g21,b21,g22,b22,g3,b3
    ssel = nc.declare_dram_parameter("ssel", [128, 32], f32, isOutput=False)
    out_B = nc.declare_dram_parameter("out_B", [T, 2, 128, HW], f32, isOutput=True)
    y_d = nc.dram_tensor("y_d", [T, B, 32, 48, 48], bf16)       # (v,u) free order
    a2m_d = nc.dram_tensor("a2m_d", [T, 2, 128, HW], bf16)
    a2p_d = nc.dram_tensor("a2p_d", [T, 2, 128, HW], bf16)
    yf_d = nc.dram_tensor("yf_d", [T, B, 32, 48, 48], bf16)     # (u,v) free order
    invN = 1.0 / NB

    def bn_finalize(wpool, stats4, gcol, scl128s, sh128s, n_streams=2):
        """stats4: [32,4] sbuf cols (sum_m, ssq_m, sum_p, ssq_p) already folded.
        Writes per-partition [128,1] scl/sh tiles per stream."""
        mean4 = wpool.tile([32, 4], f32, tag="mean4")
        nc.vector.tensor_scalar(mean4[:], stats4[:], invN, None, AOT.mult)
        for st_ in range(n_streams):
            mcol = mean4[:, 2 * st_:2 * st_ + 1]
            ecol = mean4[:, 2 * st_ + 1:2 * st_ + 2]
            m2 = wpool.tile([32, 1], f32, tag=f"m2_{st_}")
            nc.vector.tensor_tensor(m2[:], mcol, mcol, AOT.mult)
            var = wpool.tile([32, 1], f32, tag=f"var_{st_}")
            nc.vector.tensor_tensor(var[:], ecol, m2[:], AOT.subtract)
            nc.vector.tensor_scalar(var[:], var[:], EPS, None, AOT.add)
            std = wpool.tile([32, 1], f32, tag=f"std_{st_}")
            nc.scalar.activation(std[:], var[:], AFT.Sqrt)
            rstd = wpool.tile([32, 1], f32, tag=f"rstd_{st_}")
            nc.vector.reciprocal(rstd[:], std[:])
            scl32 = wpool.tile([32, 1], f32, tag=f"scl32_{st_}")
            nc.vector.tensor_tensor(scl32[:], gcol[2 * st_], rstd[:], AOT.mult)
            tmp = wpool.tile([32, 1], f32, tag=f"shtmp_{st_}")
            nc.vector.tensor_tensor(tmp[:], mcol, scl32[:], AOT.mult)
            sh32 = wpool.tile([32, 1], f32, tag=f"sh32_{st_}")
            nc.vector.tensor_tensor(sh32[:], gcol[2 * st_ + 1], tmp[:], AOT.subtract)
            for m in range(4):
                nc.sync.dma_start(scl128s[st_][32 * m:32 * m + 32, :], scl32[:])
                nc.sync.dma_start(sh128s[st_][32 * m:32 * m + 32, :], sh32[:])

    with tile.TileContext(nc) as tc:
        with tc.tile_pool(name="const", bufs=1) as cpool, \
             tc.tile_pool(name="bnsb", bufs=1) as bpool:
            nbig = cpool.tile([128, 1], f32); nc.vector.memset(nbig[:], -SBIG)
            nlam = cpool.tile([128, 1], f32); nc.vector.memset(nlam[:], -LAM)
            w1t = cpool.tile([128, 128], bf16); nc.sync.dma_start(w1t[:], w1d4[:])
            w1tn = cpool.tile([128, 128], bf16); nc.sync.dma_start(w1tn[:], w1d4n[:])
            fwr = cpool.tile([96, 96], bf16); nc.sync.dma_start(fwr[:], fwb[0])
            fwi = cpool.tile([96, 96], bf16); nc.sync.dma_start(fwi[:], fwb[1])
            fhr = cpool.tile([96, 96], bf16); nc.sync.dma_start(fhr[:], fhb[0])
            fhin = cpool.tile([96, 96], bf16); nc.sync.dma_start(fhin[:], fhb[1])
            idbt = cpool.tile([96, 96], bf16); nc.sync.dma_start(idbt[:], idb[:])
            idft = cpool.tile([96, 96], f32); nc.sync.dma_start(idft[:], idf[:])
            gbt = cpool.tile([32, 6], f32)
            nc.sync.dma_start(gbt[:], gb[:].rearrange("k c -> c k"))
            sselt = cpool.tile([128, 32], f32); nc.sync.dma_start(sselt[:], ssel[:])
            # stat collectors (col = t*2+g) and bn tiles
            coll = {}
            for nm in ("sum_m", "ssq_m", "sum_p", "ssq_p", "sum_3", "ssq_3"):
                ncol = 40 if nm in ("sum_m", "sum_p") else 8
                coltile = bpool.tile([128, ncol], f32, tag=f"coll_{nm}")
                coll[nm] = coltile
            scl2 = []
            sh2 = []
            for s in range(2):
                sctile = bpool.tile([128, 1], f32, tag=f"scl2_{s}")
                scl2.append(sctile)
                shtile = bpool.tile([128, 1], f32, tag=f"sh2_{s}")
                sh2.append(shtile)
            scl3t = bpool.tile([128, 1], f32, tag="scl3")
            sh3t = bpool.tile([128, 1], f32, tag="sh3")
            scl3 = [scl3t]
            sh3 = [sh3t]

            with tc.tile_pool(name="a2res", bufs=1) as a2pool:

                # ---------------- stage 2: BN1 affine -> LIF3 -> einsum2 + stats
                with tc.tile_pool(name="s2w", bufs=2) as wpool, \
                     tc.tile_pool(name="s2st", bufs=1) as spool, \
                     tc.tile_pool(name="s2ps", bufs=2, space="PSUM") as ps:
                    w3 = {}
                    for g in range(2):
                        for st_ in range(2):
                            wtile = spool.tile([128, HW], f32, tag=f"w3_{g}_{st_}")
                            w3[g, st_] = wtile
                            nc.vector.memset(wtile[:], 0.0)
                    for t in range(T):
                        for g in range(2):
                            col = t * 2 + g
                            o1 = []
                            for st_, dram in ((0, amb_B), (1, apb_B)):
                                at = wpool.tile([128, HW], f32, tag=f"at{st_}")
                                nc.sync.dma_start(at[:], dram[t, g])
                                w = w3[g, st_]
                                nc.vector.scalar_tensor_tensor(w[:], w[:], 0.5, at[:],
                                                               AOT.mult, AOT.add)
                                sbf = wpool.tile([128, HW], bf16, tag=f"sbf{st_}")
                                nc.scalar.activation(sbf[:], w[:], AFT.Sigmoid,
                                                     bias=nbig[:], scale=SBIG)
                                nc.vector.scalar_tensor_tensor(w[:], w[:], 1.0, w[:],
                                                               AOT.is_lt, AOT.mult)
                                o1.append(sbf)
                            a2m = wpool.tile([128, HW], bf16, tag="a2m")
                            a2p = wpool.tile([128, HW], bf16, tag="a2p")
                            off2 = 0
                            for ci, cw in enumerate(CCHUNK):
                                sl = slice(off2, off2 + cw)
                                ccol = col * 5 + ci
                                pa = ps.tile([128, 512], f32, tag="mm0")
                                nc.tensor.matmul(pa[:, :cw], w1t[:], o1[0][:, sl], start=True, stop=False)
                                nc.tensor.matmul(pa[:, :cw], w1tn[:], o1[1][:, sl], start=False, stop=True)
                                pb = ps.tile([128, 512], f32, tag="mm1")
                                nc.tensor.matmul(pb[:, :cw], w1t[:], o1[0][:, sl], start=True, stop=False)
                                nc.tensor.matmul(pb[:, :cw], w1t[:], o1[1][:, sl], start=False, stop=True)
                                nc.scalar.copy(a2m[:, sl], pa[:, :cw])
                                nc.scalar.copy(a2p[:, sl], pb[:, :cw])
                                off2 += cw
                            # stats for BN2: ssq on ScalarE, sums on VectorE
                            sqs = wpool.tile([128, HW], bf16, tag="sqs")
                            nc.scalar.activation(sqs[:], a2m[:], AFT.Square,
                                                 accum_out=coll["ssq_m"][:, col:col + 1])
                            nc.vector.tensor_reduce(coll["sum_m"][:, col:col + 1], a2m[:],
                                                    mybir.AxisListType.X, AOT.add)
                            sqs2 = wpool.tile([128, HW], bf16, tag="sqs")
                            nc.scalar.activation(sqs2[:], a2p[:], AFT.Square,
                                                 accum_out=coll["ssq_p"][:, col:col + 1])
                            nc.vector.tensor_reduce(coll["sum_p"][:, col:col + 1], a2p[:],
                                                    mybir.AxisListType.X, AOT.add)
                            nc.sync.dma_start(a2m_d[t, g], a2m[:])
                            nc.sync.dma_start(a2p_d[t, g], a2p[:])

                # ---------------- BN2 stats finalize
                with tc.tile_pool(name="bnf", bufs=1) as fpool, \
                     tc.tile_pool(name="bnfps", bufs=1, space="PSUM") as fps:
                    stats4p = fpool.tile([128, 4], f32, tag="stats4p")
                    for i, nm in enumerate(("sum_m", "ssq_m", "sum_p", "ssq_p")):
                        nc.vector.tensor_reduce(stats4p[:, i:i + 1], coll[nm][:],
                                                mybir.AxisListType.X, AOT.add)
                    pf = fps.tile([32, 4], f32, tag="fold")
                    nc.tensor.matmul(pf[:], sselt[:], stats4p[:], start=True, stop=True)
                    stats4 = fpool.tile([32, 4], f32, tag="stats4")
                    nc.vector.tensor_copy(stats4[:], pf[:])
                    gcol = [gbt[:, i:i + 1] for i in range(4)]
                    bn_finalize(fpool, stats4, gcol, scl2, sh2)

                # ---------------- stage 3: BN2 affine + combine + softshrink + FFT3
                with tc.tile_pool(name="s3w", bufs=2) as wpool, \
                     tc.tile_pool(name="s3f", bufs=2) as fw_, \
                     tc.tile_pool(name="fftps", bufs=2, space="PSUM") as fps3:
                    for t in range(T):
                        for g in range(2):
                            a2mt = wpool.tile([128, HW], bf16, tag="a2mt")
                            nc.sync.dma_start(a2mt[:], a2m_d[t, g])
                            a2pt = wpool.tile([128, HW], bf16, tag="a2pt")
                            nc.sync.dma_start(a2pt[:], a2p_d[t, g])
                            o2m = wpool.tile([128, HW], bf16, tag="o2m")
                            nc.scalar.activation(o2m[:], a2mt[:], AFT.Identity,
                                                 bias=sh2[0][:], scale=scl2[0][:])
                            o2p = wpool.tile([128, HW], bf16, tag="o2p")
                            nc.scalar.activation(o2p[:], a2pt[:], AFT.Identity,
                                                 bias=sh2[1][:], scale=scl2[1][:])
                            sdt = wpool.tile([128, HW], bf16, tag="sdt")
                            nc.sync.dma_start(sdt[:], sd_B[t, g])
                            sst = wpool.tile([128, HW], bf16, tag="sst")
                            nc.sync.dma_start(sst[:], ss_B[t, g])
                            m1 = wpool.tile([128, HW], bf16, tag="m1")
                            nc.vector.tensor_tensor(m1[:], sdt[:], o2m[:], AOT.mult)
                            m2_ = wpool.tile([128, HW], bf16, tag="m2c")
                            nc.vector.tensor_tensor(m2_[:], sst[:], o2p[:], AOT.mult)
                            yc = wpool.tile([128, HW], bf16, tag="yc")
                            nc.vector.tensor_tensor(yc[:], m1[:], m2_[:], AOT.subtract)
                            r1 = wpool.tile([128, HW], bf16, tag="r1")
                            nc.scalar.activation(r1[:], yc[:], AFT.Relu, bias=nlam[:], scale=1.0)
                            r2 = wpool.tile([128, HW], bf16, tag="r2")
                            nc.scalar.activation(r2[:], yc[:], AFT.Relu, bias=nlam[:], scale=-1.0)
                            nc.vector.tensor_tensor(yc[:], r1[:], r2[:], AOT.subtract)
                            dst = y_d[t, 4 * g:4 * g + 4].rearrange("m c v u -> (m c) (v u)")
                            nc.sync.dma_start(dst, yc[:])

                        # FFT3 per sample pair (contract v, transpose, contract u)
                        for sp in range(4):
                            yt = fw_.tile([96, 1536], bf16, tag="yt")
                            for m in range(2):
                                nc.sync.dma_start(
                                    yt[48 * m:48 * m + 48, :].rearrange(
                                        "v (c u) -> v c u", c=32),
                                    y_d[t, 2 * sp + m].rearrange("c v u -> v c u"))
                            yfs = fw_.tile([96, 1536], bf16, tag="yfs")
                            off = 0
                            for ci, cw in enumerate((480, 480, 480, 96)):
                                npairs = cw // 96
                                sl = slice(off, off + cw)
                                pw_re = fps3.tile([96, 480], f32, tag="mm0")
                                nc.tensor.matmul(pw_re[:, :cw], fwr[:], yt[:, sl], start=True, stop=True)
                                pw_im = fps3.tile([96, 480], f32, tag="mm1")
                                nc.tensor.matmul(pw_im[:, :cw], fwi[:], yt[:, sl], start=True, stop=True)
                                ywp = wpool.tile([96, 480], f32, tag="ywp")
                                ywp_v = ywp[:].bitcast(bf16).rearrange(
                                    "p (f two) -> p f two", two=2)
                                nc.scalar.copy(ywp_v[:, :cw, 0], pw_re[:, :cw])
                                nc.vector.tensor_copy(ywp_v[:, :cw, 1], pw_im[:, :cw])
                                pt2 = fps3.tile([96, 480], f32, tag="tp")
                                for j in range(npairs):
                                    js = slice(j * 96, (j + 1) * 96)
                                    nc.tensor.transpose(pt2[:, js], ywp[:, js], idft[:])
                                pt2_v = pt2[:].bitcast(bf16).rearrange(
                                    "p (f two) -> p f two", two=2)
                                ywtr = wpool.tile([96, 480], bf16, tag="ywtr")
                                nc.vector.tensor_copy(ywtr[:, :cw], pt2_v[:, :cw, 0])
                                ywti = wpool.tile([96, 480], bf16, tag="ywti")
                                nc.scalar.copy(ywti[:, :cw], pt2_v[:, :cw, 1])
                                ph = fps3.tile([96, 480], f32, tag="mmh")
                                nc.tensor.matmul(ph[:, :cw], fhr[:], ywtr[:, :cw], start=True, stop=False)
                                nc.tensor.matmul(ph[:, :cw], fhin[:], ywti[:, :cw], start=False, stop=True)
                                if ci % 2 == 0:
                                    nc.scalar.copy(yfs[:, sl], ph[:, :cw])
                                else:
                                    nc.vector.tensor_copy(yfs[:, sl], ph[:, :cw])
                                off += cw
                            srcf = yfs[:].rearrange("p (cp m v) -> p cp m v", cp=16, m=2)
                            for m in range(2):
                                dstf = yf_d[t, 2 * sp + m].rearrange(
                                    "(cp c2) u v -> (c2 u) cp v", c2=2)
                                nc.sync.dma_start(dstf, srcf[:, :, m, :])

            # ---------------- stage 4: BN3 stats + affine + out
            with tc.tile_pool(name="yfres", bufs=1) as yfpool, \
                 tc.tile_pool(name="s4w", bufs=2) as wpool4, \
                 tc.tile_pool(name="s4ps", bufs=1, space="PSUM") as fps4:
                yf_res = {}
                for t in range(T):
                    for g in range(2):
                        col = t * 2 + g
                        yft = yfpool.tile([128, HW], bf16, tag=f"yf_{t}_{g}")
                        yf_res[t, g] = yft
                        nc.sync.dma_start(
                            yft[:], yf_d[t, 4 * g:4 * g + 4].rearrange("m c u v -> (m c) (u v)"))
                        sq3 = wpool4.tile([128, HW], bf16, tag="sq3")
                        nc.scalar.activation(sq3[:], yft[:], AFT.Square,
                                             accum_out=coll["ssq_3"][:, col:col + 1])
                        nc.vector.tensor_reduce(coll["sum_3"][:, col:col + 1], yft[:],
                                                mybir.AxisListType.X, AOT.add)
                stats4p3 = wpool4.tile([128, 4], f32, tag="stats4p3")
                nc.vector.memset(stats4p3[:], 0.0)
                for i, nm in enumerate(("sum_3", "ssq_3")):
                    nc.vector.tensor_reduce(stats4p3[:, i:i + 1], coll[nm][:],
                                            mybir.AxisListType.X, AOT.add)
                pf3 = fps4.tile([32, 4], f32, tag="fold3")
                nc.tensor.matmul(pf3[:], sselt[:], stats4p3[:], start=True, stop=True)
                stats43 = wpool4.tile([32, 4], f32, tag="stats43")
                nc.vector.tensor_copy(stats43[:], pf3[:])
                gcol3 = [gbt[:, 4:5], gbt[:, 5:6]]
                bn_finalize(wpool4, stats43, gcol3, scl3, sh3, n_streams=1)
                for t in range(T):
                    for g in range(2):
                        ot = wpool4.tile([128, HW], f32, tag="ot")
                        nc.scalar.activation(ot[:], yf_res[t, g][:], AFT.Identity,
                                             bias=sh3[0][:], scale=scl3[0][:])
                        nc.sync.dma_start(out_B[t, g], ot[:])
    split_waits(nc)
    return nc


# ---------------------------------------------------------------- host glue
_NCS = {}
LAST_EXEC_NS = []
LAST_PASS_NAMES = []
LAST_PROFILES = []


def _run(name, nc, in_maps, cores):
    r = run_bass_kernel_spmd(nc, in_maps, core_ids=cores)
    LAST_PASS_NAMES.append(name)
    LAST_EXEC_NS.append(r.exec_time_ns)
    LAST_PROFILES.append(r.profile_json)
    return r.results


def _get_nc(name):
    if name not in _NCS:
        _NCS[name] = {"p1": build_pass1, "pB": build_passB}[name]()
    return _NCS[name]


def _bn_affine(sums, ssqs, gamma, beta, n):
    mu = sums / n
    var = ssqs / n - mu * mu
    scl = gamma / np.sqrt(var + EPS)
    sh = beta - mu * scl
    return scl.astype(np.float32), sh.astype(np.float32)


def kernel(x, w1, g_bn1_1, b_bn1_1, g_bn1_2, b_bn1_2, g_bn2_1, b_bn2_1,
           g_bn2_2, b_bn2_2, g_bn3, b_bn3, alpha=None):
    x = np.asarray(x, np.float32)
    w1 = np.asarray(w1, np.float32)
    gb_ = {k: np.asarray(v, np.float32) for k, v in
           dict(g11=g_bn1_1, b11=b_bn1_1, g12=g_bn1_2, b12=b_bn1_2,
                g21=g_bn2_1, b21=b_bn2_1, g22=g_bn2_2, b22=b_bn2_2,
                g3=g_bn3, b3=b_bn3).items()}
    cores = list(range(NCORES))
    n_batch = float(T * B * HW)
    LAST_EXEC_NS.clear(); LAST_PASS_NAMES.clear(); LAST_PROFILES.clear()

    fr, fi = _dft()
    fwr_hi, fwr_lo = _hilo(_diag2(fr))
    fwi_hi, fwi_lo = _hilo(_diag2(fi))
    fh_ = np.stack([_diag2(fr), _diag2(fi), _diag2(-fi)])
    idb_ = np.eye(96).astype(_BF)
    idf_ = np.eye(96, dtype=np.float32)
    w1d = np.zeros((2, 128, 128), np.float32)
    for hf in range(2):
        for kk in range(4):
            w1d[hf, kk * 32:(kk + 1) * 32, kk * 32:(kk + 1) * 32] = w1[hf * 4 + kk]
    w1hi, w1lo = _hilo(w1d)

    # ---- pass 1 (batch-sharded)
    in1 = []
    for b in cores:
        in1.append({
            "x": np.ascontiguousarray(
                x[:, b].reshape(T, 2, 64, 2, H, W).transpose(0, 3, 5, 2, 1, 4)
                .reshape(T, 96, NF)),
            "fw_hi": np.stack([fwr_hi, fwi_hi]), "fw_lo": np.stack([fwr_lo, fwi_lo]),
            "fh": fh_, "idb": idb_, "idf": idf_, "w1hi": w1hi, "w1lo": w1lo,
        })
    r1 = _run("p1", _get_nc("p1"), in1, cores)

    # ---- BN1 stats on host (f64, identical to original 4-pass flow)
    sum_m = sum(r["amb_d"].sum(axis=(0, 2), dtype=np.float64) for r in r1)
    sum_p = sum(r["apb_d"].sum(axis=(0, 2), dtype=np.float64) for r in r1)
    ssq_m = sum(np.einsum('tcs,tcs->c', r["amb_d"], r["amb_d"],
                          dtype=np.float64) for r in r1)
    ssq_p = sum(np.einsum('tcs,tcs->c', r["apb_d"], r["apb_d"],
                          dtype=np.float64) for r in r1)
    sclm, shm = _bn_affine(sum_m, ssq_m, gb_["g11"], gb_["b11"], n_batch)
    sclp, shp = _bn_affine(sum_p, ssq_p, gb_["g12"], gb_["b12"], n_batch)

    # ---- reshard to channel blocks with (v,u) free order
    def reshard(arrs, dt):
        # arrs: list over b of [T, C, HW] -> per-core-k [T, 2, 128, HW] (v,u)
        full = np.stack(arrs, axis=1)          # [T, B, C, HW]
        full = full.reshape(T, B, C, H, W).transpose(0, 1, 2, 4, 3)  # (v,u)
        out = []
        for k in range(NCORES):
            blk = full[:, :, 32 * k:32 * k + 32]            # [T, B, 32, W, H]
            blk = blk.reshape(T, 2, 4, 32, HW).reshape(T, 2, 128, HW)
            out.append(np.ascontiguousarray(blk.astype(dt)))
        return out

    amb_k = reshard([(r["amb_d"] * sclm[None, :, None] + shm[None, :, None])
                     .astype(np.float32) for r in r1], np.float32)
    apb_k = reshard([(r["apb_d"] * sclp[None, :, None] + shp[None, :, None])
                     .astype(np.float32) for r in r1], np.float32)
    sd_k = reshard([r["so_d"] for r in r1], _BF)
    ss_k = reshard([r["se_d"] for r in r1], _BF)

    ssel_ = np.zeros((128, 32), np.float32)
    for p in range(128):
        ssel_[p, p % 32] = 1.0
    fwb_ = np.stack([_diag2(fr), _diag2(fi)]).astype(_BF)
    fhb_ = np.stack([_diag2(fr), _diag2(-fi)]).astype(_BF)

    inB = []
    for k in cores:
        ch = slice(32 * k, 32 * k + 32)
        w1d4_ = np.zeros((128, 128), np.float32)
        for m in range(4):
            w1d4_[m * 32:(m + 1) * 32, m * 32:(m + 1) * 32] = w1[k]
        gbv = np.stack([gb_["g21"][ch], gb_["b21"][ch], gb_["g22"][ch],
                        gb_["b22"][ch], gb_["g3"][ch], gb_["b3"][ch]])
        inB.append({"amb_B": amb_k[k], "apb_B": apb_k[k], "sd_B": sd_k[k],
                    "ss_B": ss_k[k], "w1d4": w1d4_.astype(_BF),
                    "w1d4n": (-w1d4_).astype(_BF),
                    "fwb": fwb_, "fhb": fhb_, "idb": idb_, "idf": idf_, "gb": gbv,
                    "ssel": ssel_})
    rB = _run("pB", _get_nc("pB"), inB, cores)

    out = np.empty((T, B, C, H, W), np.float32)
    for k in cores:
        ob = rB[k]["out_B"].reshape(T, 2, 4, 32, H, W)
        for g in range(2):
            for m in range(4):
                out[:, 4 * g + m, 32 * k:32 * k + 32] = ob[:, g, m]
    return out


# revision 30
# speedup vs baseline: 1.0972x; 1.0238x over previous
"""Trainium2 Bass kernel for the spiking spectral net (nn_ASFF).

Pipeline: LIF -> FFT2 -> LIF -> blockdiag matmul -> BN -> LIF -> blockdiag
matmul -> BN -> combine -> softshrink -> FFT2.real -> BN.

Sharding: data-parallel over B (8 samples -> 8 cores). Four SPMD NEFF passes
with host-side all-reduce of BatchNorm statistics between them (stats are
[C]-vectors; everything heavy stays on device).

Layout notes:
 - c-layout: [128 partitions = half of C, 2304 free = (u,v) flattened hw]
 - spatial layout: [96 partitions = (d,h) with d = C-half, 6144 free]
 - FFT2 per 48x48 tile is done as W-side DFT (contract w), TensorE
   transpose, H-side DFT (contract h). DFT matrices are symmetric.
 - Matmuls on spike inputs use bf16 hi/lo-split DFT/weight matrices
   (exact to ~2^-17); the second FFT side has continuous input and uses
   exact fp32 matmuls. Post-threshold math (pass 3) is bf16 throughout.
"""
import sys
sys.path.insert(0, '/opt/trn_rl_repo')
import numpy as np
import ml_dtypes
import concourse.bass as bass
import concourse.tile as tile
import concourse.mybir as mybir
from concourse.bass_utils import run_bass_kernel_spmd

f32, bf16, f32r = mybir.dt.float32, mybir.dt.bfloat16, mybir.dt.float32r
AOT = mybir.AluOpType
AFT = mybir.ActivationFunctionType

T, B, C, H, W = 4, 8, 256, 48, 48
K, BS = 8, 32
HW = H * W            # 2304
NCORES = 8
NP = 96               # spatial-layout partitions (2 c-halves x 48)
NF = 6144             # spatial-layout free size (64 pairs x 2 x 48)
LAM = 0.06
EPS = 1e-5
NCHUNK = [480] * 12 + [384]           # spatial free chunking (5/4 pairs each)
CCHUNK = [512, 512, 512, 512, 256]    # c-layout free chunking of 2304

_BF = ml_dtypes.bfloat16
SBIG = float(2 ** 30)


def _hilo(x):
    hi = x.astype(np.float32).astype(_BF)
    lo = (x.astype(np.float32) - hi.astype(np.float32)).astype(_BF)
    return hi, lo


def _dft():
    j = np.arange(48)
    ang = -2.0 * np.pi * np.outer(j, j) / 48.0
    fr = (np.cos(ang) / np.sqrt(48.0)).astype(np.float32)
    fi = (np.sin(ang) / np.sqrt(48.0)).astype(np.float32)
    return fr, fi


def _diag2(m):
    out = np.zeros((96, 96), m.dtype)
    out[:48, :48] = m
    out[48:, 48:] = m
    return out


def split_waits(nc, max_waits=1):
    """This toolchain's walrus only tolerates one sync-wait per instruction;
    spill extra waits onto NoOps inserted just before the instruction."""
    ctr = 0
    for f in nc.m.functions:
        for bb in f.blocks:
            insts = list(bb.instructions)
            out = []
            changed = False
            for inst in insts:
                si = inst.sync_info
                waits = list(si.on_wait) if si else []
                if len(waits) > max_waits:
                    for wcond in waits[:-max_waits]:
                        ctr += 1
                        nop = mybir.InstNoOp(name=f"wsplit-{ctr}")
                        nop.engine = inst.engine
                        nop.sync_info = mybir.SyncInfo(on_wait=[wcond], on_update=[])
                        out.append(nop)
                    si.on_wait = waits[-max_waits:]
                    changed = True
                out.append(inst)
            if changed:
                bb.instructions = out
    return ctr


def _lif_step(nc, w_state, x_ap, s_out, ns_scratch=None):
    """One LIF step on w = 2*v scaled state: u = 0.5*w + x (into w_state),
    s = (u >= 1), w = min(u,1) - s  (== u*(u<1) bitwise).  x_ap may be PSUM."""
    nc.vector.scalar_tensor_tensor(w_state, w_state, 0.5, x_ap, AOT.mult, AOT.add)
    nc.vector.tensor_scalar(s_out, w_state, 1.0, None, AOT.is_ge)
    nc.vector.scalar_tensor_tensor(w_state, w_state, 1.0, s_out, AOT.min, AOT.subtract)


# ---------------------------------------------------------------- pass 1
def build_pass1():
    nc = bass.Bass()
    x = nc.declare_dram_parameter("x", [T, 96, NF], f32, isOutput=False)
    fw_hi = nc.declare_dram_parameter("fw_hi", [2, 96, 96], bf16, isOutput=False)
    fw_lo = nc.declare_dram_parameter("fw_lo", [2, 96, 96], bf16, isOutput=False)
    fh = nc.declare_dram_parameter("fh", [3, 96, 96], f32, isOutput=False)  # Fr2, Fi2, -Fi2
    idb = nc.declare_dram_parameter("idb", [96, 96], bf16, isOutput=False)
    idf = nc.declare_dram_parameter("idf", [96, 96], f32, isOutput=False)
    w1hi = nc.declare_dram_parameter("w1hi", [2, 128, 128], bf16, isOutput=False)
    w1lo = nc.declare_dram_parameter("w1lo", [2, 128, 128], bf16, isOutput=False)
    so_d = nc.declare_dram_parameter("so_d", [T, C, HW], bf16, isOutput=True)  # sd = so - se
    se_d = nc.declare_dram_parameter("se_d", [T, C, HW], bf16, isOutput=True)  # ss = so + se
    amb_d = nc.declare_dram_parameter("amb_d", [T, C, HW], f32, isOutput=True)
    apb_d = nc.declare_dram_parameter("apb_d", [T, C, HW], f32, isOutput=True)

    with tile.TileContext(nc) as tc:
        with tc.tile_pool(name="const", bufs=1) as cpool, \
             tc.tile_pool(name="state", bufs=1) as spool, \
             tc.tile_pool(name="work", bufs=2) as wpool, \
             tc.tile_pool(name="xtp", bufs=1) as xtp, \
             tc.tile_pool(name="ps", bufs=2, space="PSUM") as ps, \
             tc.tile_pool(name="pst", bufs=2, space="PSUM") as pst:

            fwr_hi = cpool.tile([96, 96], bf16); nc.sync.dma_start(fwr_hi[:], fw_hi[0])
            fwi_hi = cpool.tile([96, 96], bf16); nc.sync.dma_start(fwi_hi[:], fw_hi[1])
            fwr_lo = cpool.tile([96, 96], bf16); nc.sync.dma_start(fwr_lo[:], fw_lo[0])
            fwi_lo = cpool.tile([96, 96], bf16); nc.sync.dma_start(fwi_lo[:], fw_lo[1])
            fhr = cpool.tile([96, 96], f32); nc.sync.dma_start(fhr[:], fh[0])
            fhi = cpool.tile([96, 96], f32); nc.sync.dma_start(fhi[:], fh[1])
            fhin = cpool.tile([96, 96], f32); nc.sync.dma_start(fhin[:], fh[2])
            idbt = cpool.tile([96, 96], bf16); nc.sync.dma_start(idbt[:], idb[:])
            idft = cpool.tile([96, 96], f32); nc.sync.dma_start(idft[:], idf[:])
            w1t = []
            for hf in range(2):
                whi = cpool.tile([128, 128], bf16, tag=f"whi{hf}")
                nc.sync.dma_start(whi[:], w1hi[hf])
                wlo = cpool.tile([128, 128], bf16, tag=f"wlo{hf}")
                nc.sync.dma_start(wlo[:], w1lo[hf])
                w1t.append((whi, wlo))

            w1s = spool.tile([NP, NF], f32); nc.vector.memset(w1s[:], 0.0)
            wr = spool.tile([NP, NF], f32); nc.vector.memset(wr[:], 0.0)
            wi = spool.tile([NP, NF], f32); nc.vector.memset(wi[:], 0.0)

            for t in range(T):
                # ---- load x[t] in spatial layout [(d,h), (c',w)]; one DMA per d
                xt = xtp.tile([NP, NF], f32, tag="xt")
                nc.sync.dma_start(xt[:], x[t])

                # ---- fused chunk loop: LIF1 -> T1 -> W1 -> T2 -> H2 -> LIF2 -> store
                off = 0
                for g in range(13):
                    cw = NCHUNK[g]
                    npairs = cw // 96
                    sl = slice(off, off + cw)
                    # LIF1 chunk
                    s_c = wpool.tile([96, 480], bf16, tag="s_c")
                    _lif_step(nc, w1s[:, sl], xt[:, sl], s_c[:, :cw])
                    # W1 (contract w, bf16 hi/lo) directly on spikes
                    pr_ = ps.tile([96, 480], f32, tag="mm0")
                    nc.tensor.matmul(pr_[:, :cw], fwr_hi[:], s_c[:, :cw], start=True, stop=False)
                    nc.tensor.matmul(pr_[:, :cw], fwr_lo[:], s_c[:, :cw], start=False, stop=True)
                    pi_ = ps.tile([96, 480], f32, tag="mm1")
                    nc.tensor.matmul(pi_[:, :cw], fwi_hi[:], s_c[:, :cw], start=True, stop=False)
                    nc.tensor.matmul(pi_[:, :cw], fwi_lo[:], s_c[:, :cw], start=False, stop=True)
                    xw_r = wpool.tile([96, 480], f32, tag="xw_r")
                    xw_i = wpool.tile([96, 480], f32, tag="xw_i")
                    nc.scalar.copy(xw_r[:, :cw], pr_[:, :cw])
                    nc.vector.tensor_copy(xw_i[:, :cw], pi_[:, :cw])
                    # T2 back to [(d,h), ...] fp32, batched into one psum tile per tensor
                    pt2r = pst.tile([96, 480], f32, tag="tp")
                    for j in range(npairs):
                        js = slice(j * 96, (j + 1) * 96)
                        nc.tensor.transpose(pt2r[:, js], xw_r[:, js], idft[:])
                    xwtr = wpool.tile([96, 480], f32, tag="xwtr")
                    nc.scalar.copy(xwtr[:, :cw], pt2r[:, :cw])
                    pt2i = pst.tile([96, 480], f32, tag="tp")
                    for j in range(npairs):
                        js = slice(j * 96, (j + 1) * 96)
                        nc.tensor.transpose(pt2i[:, js], xw_i[:, js], idft[:])
                    xwti = wpool.tile([96, 480], f32, tag="xwti")
                    nc.vector.tensor_copy(xwti[:, :cw], pt2i[:, :cw])
                    # H2 (contract h, exact fp32) + LIF2 fused
                    pre = ps.tile([96, 480], f32, tag="mm0")
                    nc.tensor.matmul(pre[:, :cw], fhr[:], xwtr[:, :cw], start=True, stop=False)
                    nc.tensor.matmul(pre[:, :cw], fhin[:], xwti[:, :cw], start=False, stop=True)
                    pim = ps.tile([96, 480], f32, tag="mm1")
                    nc.tensor.matmul(pim[:, :cw], fhi[:], xwtr[:, :cw], start=True, stop=False)
                    nc.tensor.matmul(pim[:, :cw], fhr[:], xwti[:, :cw], start=False, stop=True)
                    so_c = wpool.tile([96, 480], bf16, tag="so_cc")
                    se_c = wpool.tile([96, 480], bf16, tag="se_cc")
                    _lif_step(nc, wr[:, sl], pre[:, :cw], so_c[:, :cw])
                    _lif_step(nc, wi[:, sl], pim[:, :cw], se_c[:, :cw])
                    sd_c = wpool.tile([96, 480], bf16, tag="sd_cc")
                    ss_c = wpool.tile([96, 480], bf16, tag="ss_cc")
                    nc.vector.tensor_tensor(sd_c[:, :cw], so_c[:, :cw], se_c[:, :cw], AOT.subtract)
                    nc.vector.tensor_tensor(ss_c[:, :cw], so_c[:, :cw], se_c[:, :cw], AOT.add)
                    # store spike-diff chunks to DRAM in [c][u][v] order
                    p0 = off // 96  # first pair index of chunk
                    for (tile_, dram) in ((sd_c, so_d), (ss_c, se_d)):
                        for d in range(2):
                            c0 = d * 128 + p0 * 2
                            dst2 = dram[t, c0:c0 + npairs * 2].rearrange(
                                "(pr ct) (u v) -> u pr ct v", ct=2, v=48)
                            src2 = tile_[d * 48:(d + 1) * 48, :cw].rearrange(
                                "u (pr ct v) -> u pr ct v", ct=2, v=48)
                            nc.sync.dma_start(dst2, src2)
                    off += cw

                # ---- einsum1 directly on sd/ss from c-layout reload
                for hf in range(2):
                    sd_r = wpool.tile([128, HW], bf16, tag="so_r", bufs=1)
                    nc.sync.dma_start(sd_r[:], so_d[t, hf * 128:(hf + 1) * 128, :])
                    ss_r = wpool.tile([128, HW], bf16, tag="se_r", bufs=1)
                    nc.sync.dma_start(ss_r[:], se_d[t, hf * 128:(hf + 1) * 128, :])
                    whi, wlo = w1t[hf]
                    off2 = 0
                    for ci, cw in enumerate(CCHUNK):
                        sl = slice(off2, off2 + cw)
                        pa = ps.tile([128, 512], f32, tag="mm0")
                        nc.tensor.matmul(pa[:, :cw], whi[:], sd_r[:, sl], start=True, stop=False)
                        nc.tensor.matmul(pa[:, :cw], wlo[:], sd_r[:, sl], start=False, stop=True)
                        pb = ps.tile([128, 512], f32, tag="mm1")
                        nc.tensor.matmul(pb[:, :cw], whi[:], ss_r[:, sl], start=True, stop=False)
                        nc.tensor.matmul(pb[:, :cw], wlo[:], ss_r[:, sl], start=False, stop=True)
                        amb = wpool.tile([128, 512], f32, tag="amb")
                        apb = wpool.tile([128, 512], f32, tag="apb")
                        nc.scalar.copy(amb[:, :cw], pa[:, :cw])
                        nc.vector.tensor_copy(apb[:, :cw], pb[:, :cw])
                        nc.sync.dma_start(amb_d[t, hf * 128:(hf + 1) * 128, sl], amb[:, :cw])
                        nc.sync.dma_start(apb_d[t, hf * 128:(hf + 1) * 128, sl], apb[:, :cw])
                        off2 += cw

    split_waits(nc)
    return nc


# ------------------------------------------------------- fused pass B (2+3+4)
# Channel-block sharded: core k owns channels [32k, 32k+32) for ALL samples.
# c-layout tiles: [128 = (4 samples x 32 ch), 2304 free].  Free order is (v,u)
# for everything before FFT3 so the spatial reload needs no pre-transpose.
NB = T * B * HW * 1.0   # per-channel count for BN stats


def build_passB():
    nc = bass.Bass()
    amb_B = nc.declare_dram_parameter("amb_B", [T, 2, 128, HW], f32, isOutput=False)
    apb_B = nc.declare_dram_parameter("apb_B", [T, 2, 128, HW], f32, isOutput=False)
    sd_B = nc.declare_dram_parameter("sd_B", [T, 2, 128, HW], bf16, isOutput=False)
    ss_B = nc.declare_dram_parameter("ss_B", [T, 2, 128, HW], bf16, isOutput=False)
    w1d4 = nc.declare_dram_parameter("w1d4", [128, 128], bf16, isOutput=False)
    w1d4n = nc.declare_dram_parameter("w1d4n", [128, 128], bf16, isOutput=False)
    fwb = nc.declare_dram_parameter("fwb", [2, 96, 96], bf16, isOutput=False)
    fhb = nc.declare_dram_parameter("fhb", [2, 96, 96], bf16, isOutput=False)
    idb = nc.declare_dram_parameter("idb", [96, 96], bf16, isOutput=False)
    idf = nc.declare_dram_parameter("idf", [96, 96], f32, isOutput=False)
    gb = nc.declare_dram_parameter("gb", [6, 32], f32, isOutput=False)  # g21,b21,g22,b22,g3,b3
    ssel = nc.declare_dram_parameter("ssel", [128, 32], f32, isOutput=False)
    sselT = nc.declare_dram_parameter("sselT", [32, 128], f32, isOutput=False)
    out_B = nc.declare_dram_parameter("out_B", [T, 2, 128, HW], f32, isOutput=True)
    y_d = nc.dram_tensor("y_d", [T, B, 32, 48, 48], bf16)       # (v,u) free order
    a2m_d = nc.dram_tensor("a2m_d", [T, 2, 128, HW], bf16)
    a2p_d = nc.dram_tensor("a2p_d", [T, 2, 128, HW], bf16)
    yf_d = nc.dram_tensor("yf_d", [T, B, 32, 48, 48], bf16)     # (u,v) free order
    invN = 1.0 / NB

    def bn_finalize(wpool, pspool, sselTt, stats4, gcol, scl128s, sh128s, n_streams=2):
        """stats4: [32,4] sbuf cols (sum_m, ssq_m, sum_p, ssq_p) already folded.
        Writes per-partition [128,1] scl/sh tiles per stream."""
        mean4 = wpool.tile([32, 4], f32, tag="mean4")
        nc.vector.tensor_scalar(mean4[:], stats4[:], invN, None, AOT.mult)
        for st_ in range(n_streams):
            mcol = mean4[:, 2 * st_:2 * st_ + 1]
            ecol = mean4[:, 2 * st_ + 1:2 * st_ + 2]
            m2 = wpool.tile([32, 1], f32, tag=f"m2_{st_}")
            nc.vector.tensor_tensor(m2[:], mcol, mcol, AOT.mult)
            var = wpool.tile([32, 1], f32, tag=f"var_{st_}")
            nc.vector.tensor_tensor(var[:], ecol, m2[:], AOT.subtract)
            nc.vector.tensor_scalar(var[:], var[:], EPS, None, AOT.add)
            std = wpool.tile([32, 1], f32, tag=f"std_{st_}")
            nc.scalar.activation(std[:], var[:], AFT.Sqrt)
            rstd = wpool.tile([32, 1], f32, tag=f"rstd_{st_}")
            nc.vector.reciprocal(rstd[:], std[:])
            scl32 = wpool.tile([32, 1], f32, tag=f"scl32_{st_}")
            nc.vector.tensor_tensor(scl32[:], gcol[2 * st_], rstd[:], AOT.mult)
            tmp = wpool.tile([32, 1], f32, tag=f"shtmp_{st_}")
            nc.vector.tensor_tensor(tmp[:], mcol, scl32[:], AOT.mult)
            sh32 = wpool.tile([32, 1], f32, tag=f"sh32_{st_}")
            nc.vector.tensor_tensor(sh32[:], gcol[2 * st_ + 1], tmp[:], AOT.subtract)
            for vec32, out128 in ((scl32, scl128s[st_]), (sh32, sh128s[st_])):
                pbc = pspool.tile([128, 1], f32, tag="bcast")
                nc.tensor.matmul(pbc[:], sselTt[:], vec32[:], start=True, stop=True)
                nc.vector.tensor_copy(out128[:], pbc[:])

    with tile.TileContext(nc) as tc:
        with tc.tile_pool(name="const", bufs=1) as cpool, \
             tc.tile_pool(name="bnsb", bufs=1) as bpool:
            nbig = cpool.tile([128, 1], f32); nc.vector.memset(nbig[:], -SBIG)
            nlam = cpool.tile([128, 1], f32); nc.vector.memset(nlam[:], -LAM)
            w1t = cpool.tile([128, 128], bf16); nc.sync.dma_start(w1t[:], w1d4[:])
            w1tn = cpool.tile([128, 128], bf16); nc.sync.dma_start(w1tn[:], w1d4n[:])
            fwr = cpool.tile([96, 96], bf16); nc.sync.dma_start(fwr[:], fwb[0])
            fwi = cpool.tile([96, 96], bf16); nc.sync.dma_start(fwi[:], fwb[1])
            fhr = cpool.tile([96, 96], bf16); nc.sync.dma_start(fhr[:], fhb[0])
            fhin = cpool.tile([96, 96], bf16); nc.sync.dma_start(fhin[:], fhb[1])
            idbt = cpool.tile([96, 96], bf16); nc.sync.dma_start(idbt[:], idb[:])
            idft = cpool.tile([96, 96], f32); nc.sync.dma_start(idft[:], idf[:])
            gbt = cpool.tile([32, 6], f32)
            nc.sync.dma_start(gbt[:], gb[:].rearrange("k c -> c k"))
            sselt = cpool.tile([128, 32], f32); nc.sync.dma_start(sselt[:], ssel[:])
            sselTt = cpool.tile([32, 128], f32); nc.sync.dma_start(sselTt[:], sselT[:])
            # stat collectors (col = t*2+g) and bn tiles
            coll = {}
            for nm in ("sum_m", "ssq_m", "sum_p", "ssq_p", "sum_3", "ssq_3"):
                ncol = 40 if nm in ("sum_m", "sum_p") else 8
                coltile = bpool.tile([128, ncol], f32, tag=f"coll_{nm}")
                coll[nm] = coltile
            scl2 = []
            sh2 = []
            for s in range(2):
                sctile = bpool.tile([128, 1], f32, tag=f"scl2_{s}")
                scl2.append(sctile)
                shtile = bpool.tile([128, 1], f32, tag=f"sh2_{s}")
                sh2.append(shtile)
            scl3t = bpool.tile([128, 1], f32, tag="scl3")
            sh3t = bpool.tile([128, 1], f32, tag="sh3")
            scl3 = [scl3t]
            sh3 = [sh3t]

            with tc.tile_pool(name="a2res", bufs=1) as a2pool:

                # ---------------- stage 2: BN1 affine -> LIF3 -> einsum2 + stats
                with tc.tile_pool(name="s2w", bufs=2) as wpool, \
                     tc.tile_pool(name="s2st", bufs=1) as spool, \
                     tc.tile_pool(name="s2ps", bufs=2, space="PSUM") as ps:
                    w3 = {}
                    for g in range(2):
                        for st_ in range(2):
                            wtile = spool.tile([128, HW], f32, tag=f"w3_{g}_{st_}")
                            w3[g, st_] = wtile
                            nc.vector.memset(wtile[:], 0.0)
                    for t in range(T):
                        for g in range(2):
                            col = t * 2 + g
                            o1 = []
                            for st_, dram in ((0, amb_B), (1, apb_B)):
                                at = wpool.tile([128, HW], f32, tag=f"at{st_}")
                                nc.sync.dma_start(at[:], dram[t, g])
                                w = w3[g, st_]
                                nc.vector.scalar_tensor_tensor(w[:], w[:], 0.5, at[:],
                                                               AOT.mult, AOT.add)
                                sbf = wpool.tile([128, HW], bf16, tag=f"sbf{st_}")
                                nc.scalar.activation(sbf[:], w[:], AFT.Sigmoid,
                                                     bias=nbig[:], scale=SBIG)
                                nc.vector.scalar_tensor_tensor(w[:], w[:], 1.0, w[:],
                                                               AOT.is_lt, AOT.mult)
                                o1.append(sbf)
                            a2m = wpool.tile([128, HW], bf16, tag="a2m")
                            a2p = wpool.tile([128, HW], bf16, tag="a2p")
                            off2 = 0
                            for ci, cw in enumerate(CCHUNK):
                                sl = slice(off2, off2 + cw)
                                ccol = col * 5 + ci
                                pa = ps.tile([128, 512], f32, tag="mm0")
                                nc.tensor.matmul(pa[:, :cw], w1t[:], o1[0][:, sl], start=True, stop=False)
                                nc.tensor.matmul(pa[:, :cw], w1tn[:], o1[1][:, sl], start=False, stop=True)
                                pb = ps.tile([128, 512], f32, tag="mm1")
                                nc.tensor.matmul(pb[:, :cw], w1t[:], o1[0][:, sl], start=True, stop=False)
                                nc.tensor.matmul(pb[:, :cw], w1t[:], o1[1][:, sl], start=False, stop=True)
                                nc.scalar.copy(a2m[:, sl], pa[:, :cw])
                                nc.scalar.copy(a2p[:, sl], pb[:, :cw])
                                off2 += cw
                            # stats for BN2: ssq on ScalarE, sums on VectorE
                            sqs = wpool.tile([128, HW], bf16, tag="sqs")
                            nc.scalar.activation(sqs[:], a2m[:], AFT.Square,
                                                 accum_out=coll["ssq_m"][:, col:col + 1])
                            nc.vector.tensor_reduce(coll["sum_m"][:, col:col + 1], a2m[:],
                                                    mybir.AxisListType.X, AOT.add)
                            sqs2 = wpool.tile([128, HW], bf16, tag="sqs")
                            nc.scalar.activation(sqs2[:], a2p[:], AFT.Square,
                                                 accum_out=coll["ssq_p"][:, col:col + 1])
                            nc.vector.tensor_reduce(coll["sum_p"][:, col:col + 1], a2p[:],
                                                    mybir.AxisListType.X, AOT.add)
                            nc.sync.dma_start(a2m_d[t, g], a2m[:])
                            nc.sync.dma_start(a2p_d[t, g], a2p[:])

                # ---------------- BN2 stats finalize
                with tc.tile_pool(name="bnf", bufs=1) as fpool, \
                     tc.tile_pool(name="bnfps", bufs=1, space="PSUM") as fps:
                    stats4p = fpool.tile([128, 4], f32, tag="stats4p")
                    for i, nm in enumerate(("sum_m", "ssq_m", "sum_p", "ssq_p")):
                        nc.vector.tensor_reduce(stats4p[:, i:i + 1], coll[nm][:],
                                                mybir.AxisListType.X, AOT.add)
                    pf = fps.tile([32, 4], f32, tag="fold")
                    nc.tensor.matmul(pf[:], sselt[:], stats4p[:], start=True, stop=True)
                    stats4 = fpool.tile([32, 4], f32, tag="stats4")
                    nc.vector.tensor_copy(stats4[:], pf[:])
                    gcol = [gbt[:, i:i + 1] for i in range(4)]
                    bn_finalize(fpool, fps, sselTt, stats4, gcol, scl2, sh2)

                # ---------------- stage 3: BN2 affine + combine + softshrink + FFT3
                with tc.tile_pool(name="s3w", bufs=2) as wpool, \
                     tc.tile_pool(name="s3f", bufs=2) as fw_, \
                     tc.tile_pool(name="fftps", bufs=2, space="PSUM") as fps3:
                    for t in range(T):
                        for g in range(2):
                            a2mt = wpool.tile([128, HW], bf16, tag="a2mt")
                            nc.sync.dma_start(a2mt[:], a2m_d[t, g])
                            a2pt = wpool.tile([128, HW], bf16, tag="a2pt")
                            nc.sync.dma_start(a2pt[:], a2p_d[t, g])
                            o2m = wpool.tile([128, HW], bf16, tag="o2m")
                            nc.scalar.activation(o2m[:], a2mt[:], AFT.Identity,
                                                 bias=sh2[0][:], scale=scl2[0][:])
                            o2p = wpool.tile([128, HW], bf16, tag="o2p")
                            nc.scalar.activation(o2p[:], a2pt[:], AFT.Identity,
                                                 bias=sh2[1][:], scale=scl2[1][:])
                            sdt = wpool.tile([128, HW], bf16, tag="sdt")
                            nc.sync.dma_start(sdt[:], sd_B[t, g])
                            sst = wpool.tile([128, HW], bf16, tag="sst")
                            nc.sync.dma_start(sst[:], ss_B[t, g])
                            m1 = wpool.tile([128, HW], bf16, tag="m1")
                            nc.vector.tensor_tensor(m1[:], sdt[:], o2m[:], AOT.mult)
                            m2_ = wpool.tile([128, HW], bf16, tag="m2c")
                            nc.vector.tensor_tensor(m2_[:], sst[:], o2p[:], AOT.mult)
                            yc = wpool.tile([128, HW], bf16, tag="yc")
                            nc.vector.tensor_tensor(yc[:], m1[:], m2_[:], AOT.subtract)
                            r1 = wpool.tile([128, HW], bf16, tag="r1")
                            nc.scalar.activation(r1[:], yc[:], AFT.Relu, bias=nlam[:], scale=1.0)
                            r2 = wpool.tile([128, HW], bf16, tag="r2")
                            nc.scalar.activation(r2[:], yc[:], AFT.Relu, bias=nlam[:], scale=-1.0)
                            nc.vector.tensor_tensor(yc[:], r1[:], r2[:], AOT.subtract)
                            dst = y_d[t, 4 * g:4 * g + 4].rearrange("m c v u -> (m c) (v u)")
                            nc.sync.dma_start(dst, yc[:])

                        # FFT3 per sample pair (contract v, transpose, contract u)
                        for sp in range(4):
                            yt = fw_.tile([96, 1536], bf16, tag="yt")
                            for m in range(2):
                                nc.sync.dma_start(
                                    yt[48 * m:48 * m + 48, :].rearrange(
                                        "v (c u) -> v c u", c=32),
                                    y_d[t, 2 * sp + m].rearrange("c v u -> v c u"))
                            yfs = fw_.tile([96, 1536], bf16, tag="yfs")
                            off = 0
                            for ci, cw in enumerate((480, 480, 480, 96)):
                                npairs = cw // 96
                                sl = slice(off, off + cw)
                                pw_re = fps3.tile([96, 480], f32, tag="mm0")
                                nc.tensor.matmul(pw_re[:, :cw], fwr[:], yt[:, sl], start=True, stop=True)
                                pw_im = fps3.tile([96, 480], f32, tag="mm1")
                                nc.tensor.matmul(pw_im[:, :cw], fwi[:], yt[:, sl], start=True, stop=True)
                                yw_r = wpool.tile([96, 480], bf16, tag="yw_r")
                                nc.scalar.copy(yw_r[:, :cw], pw_re[:, :cw])
                                yw_i = wpool.tile([96, 480], bf16, tag="yw_i")
                                nc.vector.tensor_copy(yw_i[:, :cw], pw_im[:, :cw])
                                pt2r = fps3.tile([96, 480], bf16, tag="tp")
                                for j in range(npairs):
                                    js = slice(j * 96, (j + 1) * 96)
                                    nc.tensor.transpose(pt2r[:, js], yw_r[:, js], idbt[:])
                                ywtr = wpool.tile([96, 480], bf16, tag="ywtr")
                                nc.vector.tensor_copy(ywtr[:, :cw], pt2r[:, :cw])
                                pt2i = fps3.tile([96, 480], bf16, tag="tp")
                                for j in range(npairs):
                                    js = slice(j * 96, (j + 1) * 96)
                                    nc.tensor.transpose(pt2i[:, js], yw_i[:, js], idbt[:])
                                ywti = wpool.tile([96, 480], bf16, tag="ywti")
                                nc.scalar.copy(ywti[:, :cw], pt2i[:, :cw])
                                ph = fps3.tile([96, 480], f32, tag="mmh")
                                nc.tensor.matmul(ph[:, :cw], fhr[:], ywtr[:, :cw], start=True, stop=False)
                                nc.tensor.matmul(ph[:, :cw], fhin[:], ywti[:, :cw], start=False, stop=True)
                                if ci % 2 == 0:
                                    nc.scalar.copy(yfs[:, sl], ph[:, :cw])
                                else:
                                    nc.vector.tensor_copy(yfs[:, sl], ph[:, :cw])
                                off += cw
                            srcf = yfs[:].rearrange("p (cp m v) -> p cp m v", cp=16, m=2)
                            for m in range(2):
                                dstf = yf_d[t, 2 * sp + m].rearrange(
                                    "(cp c2) u v -> (c2 u) cp v", c2=2)
                                nc.sync.dma_start(dstf, srcf[:, :, m, :])

            # ---------------- stage 4: BN3 stats + affine + out
            with tc.tile_pool(name="yfres", bufs=1) as yfpool, \
                 tc.tile_pool(name="s4w", bufs=2) as wpool4, \
                 tc.tile_pool(name="s4ps", bufs=1, space="PSUM") as fps4:
                yf_res = {}
                for t in range(T):
                    for g in range(2):
                        col = t * 2 + g
                        yft = yfpool.tile([128, HW], bf16, tag=f"yf_{t}_{g}")
                        yf_res[t, g] = yft
                        nc.sync.dma_start(
                            yft[:], yf_d[t, 4 * g:4 * g + 4].rearrange("m c u v -> (m c) (u v)"))
                        sq3 = wpool4.tile([128, HW], bf16, tag="sq3")
                        nc.scalar.activation(sq3[:], yft[:], AFT.Square,
                                             accum_out=coll["ssq_3"][:, col:col + 1])
                        nc.vector.tensor_reduce(coll["sum_3"][:, col:col + 1], yft[:],
                                                mybir.AxisListType.X, AOT.add)
                stats4p3 = wpool4.tile([128, 4], f32, tag="stats4p3")
                nc.vector.memset(stats4p3[:], 0.0)
                for i, nm in enumerate(("sum_3", "ssq_3")):
                    nc.vector.tensor_reduce(stats4p3[:, i:i + 1], coll[nm][:],
                                            mybir.AxisListType.X, AOT.add)
                pf3 = fps4.tile([32, 4], f32, tag="fold3")
                nc.tensor.matmul(pf3[:], sselt[:], stats4p3[:], start=True, stop=True)
                stats43 = wpool4.tile([32, 4], f32, tag="stats43")
                nc.vector.tensor_copy(stats43[:], pf3[:])
                gcol3 = [gbt[:, 4:5], gbt[:, 5:6]]
                bn_finalize(wpool4, fps4, sselTt, stats43, gcol3, scl3, sh3, n_streams=1)
                for t in range(T):
                    for g in range(2):
                        ot = wpool4.tile([128, HW], f32, tag="ot")
                        nc.scalar.activation(ot[:], yf_res[t, g][:], AFT.Identity,
                                             bias=sh3[0][:], scale=scl3[0][:])
                        nc.sync.dma_start(out_B[t, g], ot[:])
    split_waits(nc)
    return nc


# ---------------------------------------------------------------- host glue
_NCS = {}
LAST_EXEC_NS = []
LAST_PASS_NAMES = []
LAST_PROFILES = []


def _run(name, nc, in_maps, cores):
    r = run_bass_kernel_spmd(nc, in_maps, core_ids=cores)
    LAST_PASS_NAMES.append(name)
    LAST_EXEC_NS.append(r.exec_time_ns)
    LAST_PROFILES.append(r.profile_json)
    return r.results


def _get_nc(name):
    if name not in _NCS:
        _NCS[name] = {"p1": build_pass1, "pB": build_passB}[name]()
    return _NCS[name]


def _bn_affine(sums, ssqs, gamma, beta, n):
    mu = sums / n
    var = ssqs / n - mu * mu
    scl = gamma / np.sqrt(var + EPS)
    sh = beta - mu * scl
    return scl.astype(np.float32), sh.astype(np.float32)


def kernel(x, w1, g_bn1_1, b_bn1_1, g_bn1_2, b_bn1_2, g_bn2_1, b_bn2_1,
           g_bn2_2, b_bn2_2, g_bn3, b_bn3, alpha=None):
    x = np.asarray(x, np.float32)
    w1 = np.asarray(w1, np.float32)
    gb_ = {k: np.asarray(v, np.float32) for k, v in
           dict(g11=g_bn1_1, b11=b_bn1_1, g12=g_bn1_2, b12=b_bn1_2,
                g21=g_bn2_1, b21=b_bn2_1, g22=g_bn2_2, b22=b_bn2_2,
                g3=g_bn3, b3=b_bn3).items()}
    cores = list(range(NCORES))
    n_batch = float(T * B * HW)
    LAST_EXEC_NS.clear(); LAST_PASS_NAMES.clear(); LAST_PROFILES.clear()

    fr, fi = _dft()
    fwr_hi, fwr_lo = _hilo(_diag2(fr))
    fwi_hi, fwi_lo = _hilo(_diag2(fi))
    fh_ = np.stack([_diag2(fr), _diag2(fi), _diag2(-fi)])
    idb_ = np.eye(96).astype(_BF)
    idf_ = np.eye(96, dtype=np.float32)
    w1d = np.zeros((2, 128, 128), np.float32)
    for hf in range(2):
        for kk in range(4):
            w1d[hf, kk * 32:(kk + 1) * 32, kk * 32:(kk + 1) * 32] = w1[hf * 4 + kk]
    w1hi, w1lo = _hilo(w1d)

    # ---- pass 1 (batch-sharded)
    in1 = []
    for b in cores:
        in1.append({
            "x": np.ascontiguousarray(
                x[:, b].reshape(T, 2, 64, 2, H, W).transpose(0, 3, 5, 2, 1, 4)
                .reshape(T, 96, NF)),
            "fw_hi": np.stack([fwr_hi, fwi_hi]), "fw_lo": np.stack([fwr_lo, fwi_lo]),
            "fh": fh_, "idb": idb_, "idf": idf_, "w1hi": w1hi, "w1lo": w1lo,
        })
    r1 = _run("p1", _get_nc("p1"), in1, cores)

    # ---- BN1 stats on host (f64, identical to original 4-pass flow)
    sum_m = sum(r["amb_d"].sum(axis=(0, 2), dtype=np.float64) for r in r1)
    sum_p = sum(r["apb_d"].sum(axis=(0, 2), dtype=np.float64) for r in r1)
    ssq_m = sum(np.einsum('tcs,tcs->c', r["amb_d"], r["amb_d"],
                          dtype=np.float64) for r in r1)
    ssq_p = sum(np.einsum('tcs,tcs->c', r["apb_d"], r["apb_d"],
                          dtype=np.float64) for r in r1)
    sclm, shm = _bn_affine(sum_m, ssq_m, gb_["g11"], gb_["b11"], n_batch)
    sclp, shp = _bn_affine(sum_p, ssq_p, gb_["g12"], gb_["b12"], n_batch)

    # ---- reshard to channel blocks with (v,u) free order
    def reshard(arrs, dt):
        # arrs: list over b of [T, C, HW] -> per-core-k [T, 2, 128, HW] (v,u)
        full = np.stack(arrs, axis=1)          # [T, B, C, HW]
        full = full.reshape(T, B, C, H, W).transpose(0, 1, 2, 4, 3)  # (v,u)
        out = []
        for k in range(NCORES):
            blk = full[:, :, 32 * k:32 * k + 32]            # [T, B, 32, W, H]
            blk = blk.reshape(T, 2, 4, 32, HW).reshape(T, 2, 128, HW)
            out.append(np.ascontiguousarray(blk.astype(dt)))
        return out

    amb_k = reshard([(r["amb_d"] * sclm[None, :, None] + shm[None, :, None])
                     .astype(np.float32) for r in r1], np.float32)
    apb_k = reshard([(r["apb_d"] * sclp[None, :, None] + shp[None, :, None])
                     .astype(np.float32) for r in r1], np.float32)
    sd_k = reshard([r["so_d"] for r in r1], _BF)
    ss_k = reshard([r["se_d"] for r in r1], _BF)

    ssel_ = np.zeros((128, 32), np.float32)
    for p in range(128):
        ssel_[p, p % 32] = 1.0
    fwb_ = np.stack([_diag2(fr), _diag2(fi)]).astype(_BF)
    fhb_ = np.stack([_diag2(fr), _diag2(-fi)]).astype(_BF)

    inB = []
    for k in cores:
        ch = slice(32 * k, 32 * k + 32)
        w1d4_ = np.zeros((128, 128), np.float32)
        for m in range(4):
            w1d4_[m * 32:(m + 1) * 32, m * 32:(m + 1) * 32] = w1[k]
        gbv = np.stack([gb_["g21"][ch], gb_["b21"][ch], gb_["g22"][ch],
                        gb_["b22"][ch], gb_["g3"][ch], gb_["b3"][ch]])
        inB.append({"amb_B": amb_k[k], "apb_B": apb_k[k], "sd_B": sd_k[k],
                    "ss_B": ss_k[k], "w1d4": w1d4_.astype(_BF),
                    "w1d4n": (-w1d4_).astype(_BF),
                    "fwb": fwb_, "fhb": fhb_, "idb": idb_, "idf": idf_, "gb": gbv,
                    "ssel": ssel_, "sselT": np.ascontiguousarray(ssel_.T)})
    rB = _run("pB", _get_nc("pB"), inB, cores)

    out = np.empty((T, B, C, H, W), np.float32)
    for k in cores:
        ob = rB[k]["out_B"].reshape(T, 2, 4, 32, H, W)
        for g in range(2):
            for m in range(4):
                out[:, 4 * g + m, 32 * k:32 * k + 32] = ob[:, g, m]
    return out
